# revision 1
# baseline (speedup 1.0000x reference)
"""Trainium2 Bass kernel for nn_EnhancedBrawlerPredictionModel (B=65536).

Data-parallel over 8 NeuronCores (8192 samples/core). Host folds all params:
  - per-token q/k/v gather tables for enemy/friend self-attention (pos-emb and
    in_proj folded; v-bias and every purely additive constant is absorbed by
    the training-mode BatchNorm downstream),
  - cross-attention in_proj folded with fa/ea out_projs (32x32 mats),
  - fc1 folded per source block; map branch via one-hot matmul against a
    128x128 lhsT table,
  - counter influence via pre-masked/scaled row table
    ctab[valid*512+e] = (e!=0)*counter[e]/max(valid,1), accumulated into the
    fc3 PSUM with identity matmuls,
  - exact full-batch BN stats via two tiny AllReduces (sum, sum-of-squares).
"""

import numpy as np

import concourse.bass as bass
import concourse.bacc as bacc
import concourse.tile as tile
import concourse.mybir as mybir
from concourse.masks import make_identity

F32 = mybir.dt.float32
BF16 = mybir.dt.bfloat16
I32 = mybir.dt.int32
I16 = mybir.dt.int16

B_FULL = 65536
NCORES = 8
E, NH, DH, S = 32, 4, 8, 3
NB, NM, H = 512, 128, 128
AOP = mybir.AluOpType
AFT = mybir.ActivationFunctionType

# ---------------------------------------------------------------------------
# host-side precompute
# ---------------------------------------------------------------------------


def host_precompute(inp):
    f32 = np.float32
    emb = np.asarray(inp['brawler_emb'], f32)
    pos_w = np.asarray(inp['pos_w'], f32)
    pos_b = np.asarray(inp['pos_b'], f32)
    pos_emb = np.arange(S, dtype=f32)[:, None] * pos_w[None, :, 0] + pos_b

    def split_in(w, b):
        w = np.asarray(w, f32)
        b = np.asarray(b, f32)
        return (w[:E], w[E:2 * E], w[2 * E:], b[:E], b[E:2 * E], b[2 * E:])

    Wq_ea, Wk_ea, Wv_ea, bq_ea, bk_ea, bv_ea = split_in(inp['ea_in_w'], inp['ea_in_b'])
    Wq_fa, Wk_fa, Wv_fa, bq_fa, bk_fa, bv_fa = split_in(inp['fa_in_w'], inp['fa_in_b'])
    Wq_ca, Wk_ca, Wv_ca, bq_ca, bk_ca, bv_ca = split_in(inp['ca_in_w'], inp['ca_in_b'])
    Wout_ea, bout_ea = np.asarray(inp['ea_out_w'], f32), np.asarray(inp['ea_out_b'], f32)
    Wout_fa, bout_fa = np.asarray(inp['fa_out_w'], f32), np.asarray(inp['fa_out_b'], f32)
    Wout_ca, bout_ca = np.asarray(inp['ca_out_w'], f32), np.asarray(inp['ca_out_b'], f32)

    t_ea = emb[None, :, :] + pos_emb[:, None, :]
    zpad_e = np.zeros((S, NB, 32), f32)
    ea_tab = np.concatenate([t_ea @ Wq_ea.T + bq_ea,
                             t_ea @ Wk_ea.T + bk_ea,
                             t_ea @ Wv_ea.T, zpad_e], -1).reshape(S * NB, 4 * E)
    zpad_f = np.zeros((NB, 32), f32)
    fa_tab = np.concatenate([emb @ Wq_fa.T + bq_fa,
                             emb @ Wk_fa.T + bk_fa,
                             emb @ Wv_fa.T, zpad_f], -1)

    Mq = Wq_ca @ Wout_fa
    bq_f = Mq @ bv_fa + Wq_ca @ bout_fa + bq_ca
    Mk = Wk_ca @ Wout_ea
    bk_f = Mk @ bv_ea + Wk_ca @ bout_ea + bk_ca
    Mv = Wv_ca @ Wout_ea

    fc1_w = np.asarray(inp['fc1_w'], f32)
    A_ca = fc1_w[:, 0:96].reshape(H, 3, E)
    A_ea = fc1_w[:, 96:192].reshape(H, 3, E)
    A_m = fc1_w[:, 192:224]
    CT = np.stack([(A_ca[:, i] @ Wout_ca).T for i in range(3)])   # (3, 32, 128)
    BT = np.stack([(A_ea[:, i] @ Wout_ea).T for i in range(3)])
    m_tab = np.asarray(inp['map_emb'], f32) @ A_m.T               # (128, 128)

    counter = np.asarray(inp['counter_matrix'], f32)
    nz = (np.arange(NB) != 0).astype(f32)[:, None]
    ctab = np.concatenate([nz * counter / max(v, 1) for v in range(4)], 0)

    W3aug = np.concatenate([np.asarray(inp['fc3_w'], f32).T,
                            np.asarray(inp['fc3_b'], f32)[None, :]], 0)

    cb = np.zeros((3, 32, 2, 128), f32)
    for i in range(3):
        cb[i, :, 0] = CT[i]
        cb[i, :, 1] = BT[i]
    return dict(
        ea_tab=ea_tab, fa_tab=fa_tab, ctab=ctab, m_tab=m_tab,
        # (96, 3, 32): [32i+k, which(q/k/v), out] - M.T replicated per token base
        mqkvT=np.tile(np.stack([Mq.T, Mk.T, Mv.T], 1), (3, 1, 1)),
        # (96, 2): per-partition bias columns for q/k (tiled over 3 tokens)
        bqk=np.stack([np.tile(bq_f, 3), np.tile(bk_f, 3)], 1),
        # (96, 2, 128): [32i+k, which(C/B), f1]
        cbT=cb.reshape(96, 2, 128),
        w2T=np.asarray(inp['fc2_w'], f32).T,
        w3aug=W3aug,
        bn1_g=np.asarray(inp['bn1_g'], f32), bn1_b=np.asarray(inp['bn1_b'], f32),
        bn2_g=np.asarray(inp['bn2_g'], f32), bn2_b=np.asarray(inp['bn2_b'], f32),
    )


def wrap_idx16(flat):
    """dma_gather index layout: (128, ceil(n/16)) int16, idx i at
    [i % 16, i // 16], replicated down the 8 16-partition groups."""
    n = len(flat)
    ncol = (n + 15) // 16
    pad = np.full(ncol * 16, -1, np.int64)
    pad[:n] = flat
    t = pad.reshape(ncol, 16).T.astype(np.int16)
    return np.tile(t, (8, 1))


# ---------------------------------------------------------------------------
# device kernel
# ---------------------------------------------------------------------------


def _attn(nc, pool, x, G, layout, out_ao):
    """Batch-major 3-token 4-head attention.
    layout 'A': x (128, G, 3, 96) token-major rows [q|k|v] (gathered tables).
    layout 'B': x (128, G, 288) = [q(3,32) | k(3,32) | v(3,32)].
    out_ao: (128, G, 3, 32) bf16, attention output pre-out_proj (v-bias-free).
    """
    if layout == 'A':
        qa = x[:, :, :, 0:32]

        def k_b(j):
            return x[:, :, j:j + 1, 32:64].to_broadcast([128, G, 3, 32])

        def v_i(j, i):
            return x[:, :, j, 64:96].rearrange("p g (h d) -> p g h d", d=DH)
    else:
        qa = x[:, :, 0:96].rearrange("p g (i d) -> p g i d", d=32)

        def k_b(j):
            return x[:, :, 96 + j * 32:96 + (j + 1) * 32].unsqueeze(2).to_broadcast(
                [128, G, 3, 32])

        def v_i(j, i):
            return x[:, :, 192 + j * 32:192 + (j + 1) * 32].rearrange(
                "p g (h d) -> p g h d", d=DH)

    M = pool.tile([128, G, 3, 3, E], BF16, tag="at_m")        # (g, j, i, d32)
    for j in range(3):
        nc.vector.tensor_tensor(out=M[:, :, j], in0=qa, in1=k_b(j), op=AOP.mult)
    # head-sum over d=8 via a strided add tree; (j,i,h) merge to one dim of 36
    M4 = M.rearrange("p g j i (h d) -> p g (j i h) d", d=DH)  # (128,G,36,8)
    t1 = pool.tile([128, G, 36, 4], BF16, tag="at_t1")
    nc.vector.tensor_tensor(out=t1, in0=M4[:, :, :, 0:4], in1=M4[:, :, :, 4:8],
                            op=AOP.add)
    t2 = pool.tile([128, G, 36, 2], BF16, tag="at_t2")
    nc.vector.tensor_tensor(out=t2, in0=t1[:, :, :, 0:2], in1=t1[:, :, :, 2:4],
                            op=AOP.add)
    s = pool.tile([128, G, 3, 3, NH], F32, tag="at_s")        # (j, i, h)
    nc.vector.tensor_tensor(out=s.rearrange("p g j i h -> p g (j i h)"),
                            in0=t2[:, :, :, 0], in1=t2[:, :, :, 1], op=AOP.add)
    e = pool.tile([128, G, 3, 3, NH], F32, tag="at_e")
    nc.scalar.activation(out=e, in_=s, func=AFT.Exp,
                         scale=float(1.0 / np.sqrt(DH)))
    den = pool.tile([128, G, 3, NH], F32, tag="at_den")       # (i, h)
    nc.vector.tensor_tensor(out=den, in0=e[:, :, 0], in1=e[:, :, 1], op=AOP.add)
    den2 = pool.tile([128, G, 3, NH], F32, tag="at_den2")
    nc.vector.tensor_tensor(out=den2, in0=den, in1=e[:, :, 2], op=AOP.add)
    r = pool.tile([128, G, 3, NH], F32, tag="at_r")
    rs = pool.tile([128, G, 3, NH], F32, tag="at_rs")
    nc.vector.reciprocal_approx_accurate(
        out=r.rearrange("p g i h -> p (g i h)"),
        in_=den2.rearrange("p g i h -> p (g i h)"),
        scratch=rs.rearrange("p g i h -> p (g i h)"))
    a = pool.tile([128, G, 3, 3, NH], BF16, tag="at_a")       # (j, i, h)
    nc.vector.tensor_tensor(
        out=a, in0=e, in1=r.unsqueeze(2).to_broadcast([128, G, 3, 3, NH]),
        op=AOP.mult)
    # AV: ao[i,h,d] = sum_j a[j,i,h] * v[j,h,d]; per (j,i): (G, 4, 8) ops.
    # No in-place accumulation (out must not alias an input on HW).
    ao_h = out_ao.rearrange("p g i (h d) -> p g i h d", d=DH)
    av0 = pool.tile([128, G, NH, DH], BF16, tag="at_av0")
    av1 = pool.tile([128, G, NH, DH], BF16, tag="at_av1")
    av2 = pool.tile([128, G, NH, DH], BF16, tag="at_av2")
    for i in range(3):
        for j, dst in ((0, av0), (1, av1), (2, av2)):
            a_b = a[:, :, j, i].unsqueeze(3).to_broadcast([128, G, NH, DH])
            nc.vector.tensor_tensor(out=dst[...], in0=a_b, in1=v_i(j, i),
                                    op=AOP.mult)
        nc.vector.tensor_tensor(out=av0[...], in0=av0[...], in1=av1[...],
                                op=AOP.add) if False else None
        s01 = pool.tile([128, G, NH, DH], BF16, tag="at_s01")
        nc.vector.tensor_tensor(out=s01[...], in0=av0[...], in1=av1[...],
                                op=AOP.add)
        nc.vector.tensor_tensor(out=ao_h[:, :, i], in0=s01[...], in1=av2[...],
                                op=AOP.add)


def build_nc(b, n_cores, with_collective=True, stage=5):
    pass
    assert b % 512 == 0
    nc = bacc.Bacc("TRN2", target_bir_lowering=False, debug=False,
                   num_devices=n_cores)

    G1 = min(8, b // 128)      # P1 chunk = 1024 samples
    CH1 = G1 * 128
    NCH1 = b // CH1
    G3 = min(4, b // 128)      # P3 chunk = 512 samples
    CH3 = G3 * 128
    NCH3 = b // CH3
    btot = float(b * (n_cores if with_collective else 1))

    dt_i = nc.dram_tensor
    eidx = dt_i("eidx", (128, 3 * b // 16), I16, kind="ExternalInput")
    fidx = dt_i("fidx", (128, 3 * b // 16), I16, kind="ExternalInput")
    cidx = dt_i("cidx", (128, 3 * b // 16), I16, kind="ExternalInput")
    midx = dt_i("midx", (1, b), I32, kind="ExternalInput")
    ea_tab = dt_i("ea_tab", (S * NB, 128), BF16, kind="ExternalInput")
    fa_tab = dt_i("fa_tab", (NB, 128), BF16, kind="ExternalInput")
    ctab = dt_i("ctab", (4 * NB, NB), BF16, kind="ExternalInput")
    m_tab = dt_i("m_tab", (NM, 128), BF16, kind="ExternalInput")
    mqkvT = dt_i("mqkvT", (96, 3, 32), BF16, kind="ExternalInput")
    bqk = dt_i("bqk", (96, 2), F32, kind="ExternalInput")
    cbT = dt_i("cbT", (96, 2, 128), BF16, kind="ExternalInput")
    w2T = dt_i("w2T", (128, 64), BF16, kind="ExternalInput")
    w3aug = dt_i("w3aug", (65, NB), BF16, kind="ExternalInput")
    bn_g1 = dt_i("bn_g1", (H, 1), F32, kind="ExternalInput")
    bn_b1 = dt_i("bn_b1", (H, 1), F32, kind="ExternalInput")
    bn_g2 = dt_i("bn_g2", (64, 1), F32, kind="ExternalInput")
    bn_b2 = dt_i("bn_b2", (64, 1), F32, kind="ExternalInput")
    out_t = dt_i("out", (b, NB), F32, kind="ExternalOutput")

    import contextlib
    with tile.TileContext(nc) as tc, contextlib.ExitStack() as ctx:
        singles = ctx.enter_context(tc.tile_pool(name="singles", bufs=1))
        dram = ctx.enter_context(tc.tile_pool(name="dram", bufs=1, space="DRAM"))

        # --- constants -----------------------------------------------------
        ident = singles.tile([128, 128], BF16)
        make_identity(nc, ident[:, :])

        def load(name, shape, dtype, src):
            t = singles.tile(shape, dtype, tag="c_" + name)
            nc.sync.dma_start(out=t[...], in_=src)
            return t

        idx_e = load("idx_e", [128, 3 * b // 16], I16, eidx[:, :])
        idx_f = load("idx_f", [128, 3 * b // 16], I16, fidx[:, :])
        idx_c = load("idx_c", [128, 3 * b // 16], I16, cidx[:, :])
        c_mqkvT = load("mqkvT", [96, 3, 32], BF16, mqkvT[:, :, :])
        c_bqk = load("bqk", [96, 2], F32, bqk[:, :])
        c_cbT = load("cbT", [96, 2, 128], BF16, cbT[:, :, :])
        c_mtab = load("mtab", [NM, 128], BF16, m_tab[:, :])
        c_w2T = load("w2T", [128, 64], BF16, w2T[:, :])
        c_w3aug = load("w3aug", [65, NB], BF16, w3aug[:, :])
        c_g1 = load("g1", [H, 1], F32, bn_g1[:, :])
        c_b1 = load("b1", [H, 1], F32, bn_b1[:, :])
        c_g2 = load("g2", [64, 1], F32, bn_g2[:, :])
        c_b2 = load("b2", [64, 1], F32, bn_b2[:, :])
        iota_c = singles.tile([128, 1], I32)
        nc.gpsimd.iota(iota_c[:, :], pattern=[[0, 1]], base=0, channel_multiplier=1)

        h1 = singles.tile([128, b], BF16)
        a1 = singles.tile([128, b], BF16)
        h2 = singles.tile([64, b], BF16)
        a2aug = singles.tile([65, b], BF16)
        nc.vector.memset(a2aug[64:65, :], 1.0)

        # --- P1: attention chain + h1 --------------------------------------
        with tc.tile_pool(name="attn", bufs=2) as atp, \
             tc.tile_pool(name="gath", bufs=3) as gath, \
             tc.tile_pool(name="ao", bufs=2) as aopool, \
             tc.tile_pool(name="stag", bufs=2) as stag, \
             tc.tile_pool(name="mp", bufs=2) as mpool, \
             tc.tile_pool(name="ps_t", bufs=1, space="PSUM") as ps_t, \
             tc.tile_pool(name="ps_proj", bufs=2, space="PSUM") as ps_proj, \
             tc.tile_pool(name="ps_xc", bufs=1, space="PSUM") as ps_xc, \
             tc.tile_pool(name="ps_h1", bufs=1, space="PSUM") as ps_h1:
            for ch in range(NCH1):
                ic0 = ch * (3 * CH1 // 16)
                ic1 = ic0 + 3 * CH1 // 16
                # dma_gather is limited to ~1024 idxs/call (SWDGE ring)
                xe = gath.tile([128, G1, 3, 128], BF16, tag="xe")
                xf = gath.tile([128, G1, 3, 128], BF16, tag="xf")
                nsub = (3 * CH1 + 1023) // 1024
                per = 3 * CH1 // nsub
                assert per % 128 == 0 and per * nsub == 3 * CH1
                for su in range(nsub):
                    xev = xe[...].rearrange("p g t e -> p (g t) e")
                    xfv = xf[...].rearrange("p g t e -> p (g t) e")
                    r0 = su * (per // 128)
                    r1 = r0 + per // 128
                    c0 = ic0 + su * (per // 16)
                    c1 = c0 + per // 16
                    nc.gpsimd.dma_gather(
                        xev[:, r0:r1, :], ea_tab[:, :], idx_e[:, c0:c1],
                        per, per, 128)
                    nc.gpsimd.dma_gather(
                        xfv[:, r0:r1, :], fa_tab[:, :], idx_f[:, c0:c1],
                        per, per, 128)

                if stage == 1:
                    nc.scalar.activation(
                        out=h1[:, ch * G1 * 64:(ch + 1) * G1 * 64].rearrange(
                            "p (g e) -> p g e", e=64),
                        in_=xe[:, :, 0, 0:64], func=AFT.Copy)
                    continue
                ao_e = aopool.tile([128, G1, 3, 32], BF16, tag="ao_e")
                ao_f = aopool.tile([128, G1, 3, 32], BF16, tag="ao_f")
                _attn(nc, atp, xe, G1, 'A', ao_e)
                _attn(nc, atp, xf, G1, 'A', ao_f)

                if stage == 2:
                    nc.vector.tensor_tensor(
                        out=h1[:, ch * G1 * 96:(ch + 1) * G1 * 96].rearrange(
                            "p (g e) -> p g e", e=96),
                        in0=ao_e[...].rearrange("p g i d -> p g (i d)"),
                        in1=ao_f[...].rearrange("p g i d -> p g (i d)"),
                        op=AOP.add)
                    continue
                for sc in range(G1 // 4):          # 512-sample sub-chunks
                    g0 = sc * 4
                    aofT_ps = ps_t.tile([96, 512], BF16, tag="aofT")
                    aoeT_ps = ps_t.tile([96, 512], BF16, tag="aoeT")
                    for t in range(4):
                        nc.tensor.transpose(
                            aofT_ps[:, t * 128:(t + 1) * 128],
                            ao_f[:, g0 + t].rearrange("p i d -> p (i d)"),
                            ident[:, :])
                        nc.tensor.transpose(
                            aoeT_ps[:, t * 128:(t + 1) * 128],
                            ao_e[:, g0 + t].rearrange("p i d -> p (i d)"),
                            ident[:, :])
                    aofT = stag.tile([96, 512], BF16, tag="aofT_s")
                    aoeT = stag.tile([96, 512], BF16, tag="aoeT_s")
                    nc.scalar.activation(out=aofT[...], in_=aofT_ps[...], func=AFT.Copy)
                    nc.scalar.activation(out=aoeT[...], in_=aoeT_ps[...], func=AFT.Copy)
                    col0 = (ch * (G1 // 4) + sc) * 512
                    if stage == 31:
                        nc.vector.tensor_tensor(
                            out=h1[0:96, col0:col0 + 512], in0=aofT[...],
                            in1=aoeT[...], op=AOP.add)
                        continue

                    # ca projections, feature-major
                    qkvT = stag.tile([96, 3, 512], BF16, tag="qkvT_s")
                    for w in range(3 if stage != 31 else 0):
                        src = aofT if w == 0 else aoeT
                        pw = ps_proj.tile([96, 512], F32, tag="projT")
                        for i in range(3):
                            sl = slice(i * 32, (i + 1) * 32)
                            nc.tensor.matmul(pw[sl, :], c_mqkvT[sl, w, :], src[sl, :],
                                             start=True, stop=True,
                                             tile_position=(32 * i, 32 * i))
                        if w < 2:
                            nc.vector.tensor_scalar(
                                out=qkvT[:, w], in0=pw[...],
                                scalar1=c_bqk[:, w:w + 1], scalar2=None,
                                op0=AOP.add)
                        else:
                            nc.scalar.activation(out=qkvT[:, w], in_=pw[...],
                                                 func=AFT.Copy)

                    if stage == 32:
                        nc.vector.tensor_tensor(
                            out=h1[0:96, col0:col0 + 512],
                            in0=qkvT[:, 1], in1=qkvT[:, 2], op=AOP.add)
                        continue
                    # back to batch-major: per group [q(3,32)|k(3,32)|v(3,32)],
                    # groups padded to 512 elems for psum bank alignment
                    xc_ps = ps_xc.tile([128, 4, 512], BF16, tag="xc_ps")
                    for t in range(4):
                        for w in range(3):
                            nc.tensor.transpose(
                                xc_ps[:, t, w * 96:(w + 1) * 96],
                                qkvT[:, w, t * 128:(t + 1) * 128],
                                ident[0:96, 0:96])
                    xc = mpool.tile([128, 4, 288], BF16, tag="xc")
                    nc.scalar.activation(out=xc[...], in_=xc_ps[:, :, 0:288],
                                         func=AFT.Copy)

                    if stage == 33:
                        nc.scalar.activation(
                            out=h1[:, col0:col0 + 288].rearrange(
                                "p (a c) -> p a c", c=288),
                            in_=xc[...][:, 0:1, :], func=AFT.Copy)
                        continue
                    att_c = aopool.tile([128, 4, 3, 32], BF16, tag="att_c")
                    _attn(nc, atp, xc, 4, 'B', att_c)

                    actT_ps = ps_t.tile([96, 512], BF16, tag="actT")
                    for t in range(4):
                        nc.tensor.transpose(
                            actT_ps[:, t * 128:(t + 1) * 128],
                            att_c[:, t].rearrange("p i d -> p (i d)"),
                            ident[:, :])
                    actT = stag.tile([96, 512], BF16, tag="actT_s")
                    nc.scalar.activation(out=actT[...], in_=actT_ps[...], func=AFT.Copy)
                    if stage == 34:
                        nc.vector.tensor_tensor(
                            out=h1[0:96, col0:col0 + 512], in0=actT[...],
                            in1=aoeT[...], op=AOP.add)
                        continue

                    # map one-hot for this 512-chunk
                    mrep = mpool.tile([128, 512], I32, tag="mrep")
                    nc.sync.dma_start(
                        out=mrep[...],
                        in_=midx[0:1, col0:col0 + 512].to_broadcast([128, 512]))
                    oh = mpool.tile([128, 512], BF16, tag="oh")
                    nc.vector.tensor_tensor(
                        out=oh[...], in0=mrep[...],
                        in1=iota_c[:, 0:1].to_broadcast([128, 512]), op=AOP.is_equal)

                    # h1 += sum_i C_i.T@att_ca_i + sum_i B_i.T@ao_e_i + m_tab@oh.
    # The per-token sums fold into single K=96 matmuls (cbT rows
    # are [C0.T;C1.T;C2.T] / [B0.T;B1.T;B2.T]).
                    h1_ps = ps_h1.tile([128, 512], F32, tag="h1ps")
                    nc.tensor.matmul(h1_ps[...], c_cbT[:, 0, :], actT[...],
                                     start=True, stop=False)
                    nc.tensor.matmul(h1_ps[...], c_cbT[:, 1, :], aoeT[...],
                                     start=False, stop=False)
                    nc.tensor.matmul(h1_ps[...], c_mtab[:, :], oh[...],
                                     start=False, stop=True)
                    nc.scalar.activation(out=h1[:, col0:col0 + 512], in_=h1_ps[...],
                                         func=AFT.Copy)

        # --- BN (exact global stats) ---------------------------------------
        def bn_stats_apply(src, parts, g_col, b_col, relu_out, cc_name):
            s1 = singles.tile([parts, 1], F32, tag=cc_name + "_s1")
            nc.vector.tensor_reduce(out=s1[...], in_=src, axis=mybir.AxisListType.X,
                                    op=AOP.add)
            q1 = singles.tile([parts, 1], F32, tag=cc_name + "_q1")
            nc.scalar.activation(out=relu_out, in_=src, func=AFT.Square,
                                 accum_out=q1[...])
            if with_collective:
                cc_in = dram.tile([parts, 2], F32, tag=cc_name + "_in")
                cc_out = nc.dram_tensor(cc_name + "_out", (parts, 2), F32,
                                        kind="Internal", addr_space="Shared")
                nc.sync.dma_start(out=cc_in[:, 0:1], in_=s1[...])
                nc.sync.dma_start(out=cc_in[:, 1:2], in_=q1[...])
                nc.gpsimd.collective_compute(
                    "AllReduce", AOP.add,
                    replica_groups=[list(range(n_cores))],
                    ins=[cc_in[:, :].opt()], outs=[cc_out[:, :].opt()])
                sq = singles.tile([parts, 2], F32, tag=cc_name + "_sq")
                nc.sync.dma_start(out=sq[...], in_=cc_out[:, :])
                s1g, q1g = sq[:, 0:1], sq[:, 1:2]
            else:
                s1g, q1g = s1[:, :], q1[:, :]
            mean = singles.tile([parts, 1], F32, tag=cc_name + "_mean")
            nc.vector.tensor_scalar_mul(mean[...], s1g, 1.0 / btot)
            msq = singles.tile([parts, 1], F32, tag=cc_name + "_msq")
            nc.vector.tensor_scalar_mul(msq[...], q1g, 1.0 / btot)
            m2 = singles.tile([parts, 1], F32, tag=cc_name + "_m2")
            nc.vector.tensor_tensor(out=m2[...], in0=mean[...], in1=mean[...],
                                    op=AOP.mult)
            var = singles.tile([parts, 1], F32, tag=cc_name + "_var")
            nc.vector.tensor_tensor(out=var[...], in0=msq[...], in1=m2[...],
                                    op=AOP.subtract)
            eps = singles.tile([parts, 1], F32, tag=cc_name + "_eps")
            nc.vector.memset(eps[...], 1e-5)
            std = singles.tile([parts, 1], F32, tag=cc_name + "_std")
            nc.scalar.activation(out=std[...], in_=var[...], func=AFT.Sqrt,
                                 bias=eps[...])
            rstd = singles.tile([parts, 1], F32, tag=cc_name + "_rstd")
            nc.vector.reciprocal(out=rstd[...], in_=std[...])
            scale = singles.tile([parts, 1], F32, tag=cc_name + "_scale")
            nc.vector.tensor_tensor(out=scale[...], in0=g_col[...], in1=rstd[...],
                                    op=AOP.mult)
            mscale = singles.tile([parts, 1], F32, tag=cc_name + "_ms")
            nc.vector.tensor_tensor(out=mscale[...], in0=mean[...], in1=scale[...],
                                    op=AOP.mult)
            nbias = singles.tile([parts, 1], F32, tag=cc_name + "_nb")
            nc.vector.tensor_tensor(out=nbias[...], in0=b_col[...], in1=mscale[...],
                                    op=AOP.subtract)
            nc.scalar.activation(out=relu_out, in_=src, func=AFT.Relu,
                                 bias=nbias[...], scale=scale[...])

        if stage <= 3 or stage in (35, 36):
            with tc.tile_pool(name="dump", bufs=2) as dump:
                out_r0 = out_t[:, :].rearrange("(g p) n -> p g n", p=128)
                for ch in range(b // 512):
                    dtile = dump.tile([128, 4, 512], F32, tag="d")
                    nc.scalar.activation(
                        out=dtile[...].rearrange("p a c -> p (a c)"),
                        in_=h1[:, 0:2048], func=AFT.Copy)
                    nc.sync.dma_start(out=out_r0[:, ch * 4:(ch + 1) * 4, :],
                                      in_=dtile[...])

        if stage >= 4 and stage not in (35, 36):
            bn_stats_apply(h1[:, :], 128, c_g1, c_b1, a1[:, :], "cc1")

        with tc.tile_pool(name="ps_h2", bufs=2, space="PSUM") as ps_h2:
            for sc in range(b // 512 if (stage >= 4 and stage not in (35, 36)) else 0):
                h2_ps = ps_h2.tile([64, 512], F32, tag="h2ps")
                nc.tensor.matmul(h2_ps[...], c_w2T[:, :],
                                 a1[:, sc * 512:(sc + 1) * 512],
                                 start=True, stop=True)
                nc.scalar.activation(out=h2[:, sc * 512:(sc + 1) * 512],
                                     in_=h2_ps[...], func=AFT.Copy)

        if stage >= 4 and stage not in (35, 36):
            bn_stats_apply(h2[:, :], 64, c_g2, c_b2, a2aug[0:64, :], "cc2")

        # --- P3: fc3 + counter + writeback ---------------------------------
        with tc.tile_pool(name="ps_o", bufs=4, space="PSUM") as ps_o, \
             tc.tile_pool(name="ct", bufs=3) as ctpool, \
             tc.tile_pool(name="ost", bufs=2) as ost:
            out_r = out_t[:, :].rearrange("(g p) n -> p g n", p=128)
            for ch in range(NCH3 if (stage >= 5 and stage not in (35, 36)) else 0):
                ic0 = ch * (3 * CH3 // 16)
                ic1 = ic0 + 3 * CH3 // 16
                ct = ctpool.tile([128, G3, 3, NB], BF16, tag="ct")
                nsub3 = (3 * CH3 + 1023) // 1024
                per3 = 3 * CH3 // nsub3
                assert per3 % 128 == 0 and per3 * nsub3 == 3 * CH3
                for su in range(nsub3):
                    ctv = ct[...].rearrange("p g t e -> p (g t) e")
                    r0 = su * (per3 // 128)
                    r1 = r0 + per3 // 128
                    c0 = ic0 + su * (per3 // 16)
                    c1 = c0 + per3 // 16
                    nc.gpsimd.dma_gather(
                        ctv[:, r0:r1, :], ctab[:, :], idx_c[:, c0:c1],
                        per3, per3, NB)
                ostg = ost.tile([128, G3, NB], F32, tag="ostg")
                for g in range(G3):
                    col0 = ch * CH3 + g * 128
                    o_ps = ps_o.tile([128, NB], F32, tag="ops")
                    nc.tensor.matmul(o_ps[...], a2aug[:, col0:col0 + 128],
                                     c_w3aug[:, :], start=True, stop=False)
                    for j in range(3):
                        nc.tensor.matmul(o_ps[...], ident[:, :], ct[:, g, j, :],
                                         start=False, stop=(j == 2))
                    nc.scalar.activation(out=ostg[:, g], in_=o_ps[...], func=AFT.Copy)
                nc.sync.dma_start(out=out_r[:, ch * G3:(ch + 1) * G3, :],
                                  in_=ostg[...])

    nc.compile()
    return nc


# ---------------------------------------------------------------------------
# host wrapper
# ---------------------------------------------------------------------------

_NC_CACHE = {}


def make_core_inputs(inputs, pc, b, n_cores):
    import ml_dtypes
    bf16 = ml_dtypes.bfloat16
    friends = np.asarray(inputs['friends'], np.int64)
    enemies = np.asarray(inputs['enemies'], np.int64)
    map_idx = np.asarray(inputs['map_idx'], np.int64)

    valid = (enemies != 0).sum(1)
    cfull = valid[:, None] * NB + enemies
    efull = np.arange(3)[None, :] * NB + enemies

    shared = dict(
        ea_tab=pc['ea_tab'].astype(bf16), fa_tab=pc['fa_tab'].astype(bf16),
        ctab=pc['ctab'].astype(bf16), m_tab=pc['m_tab'].astype(bf16),
        mqkvT=pc['mqkvT'].astype(bf16), bqk=pc['bqk'].astype(np.float32),
        cbT=pc['cbT'].astype(bf16), w2T=pc['w2T'].astype(bf16),
        w3aug=pc['w3aug'].astype(bf16),
        bn_g1=pc['bn1_g'].reshape(-1, 1).astype(np.float32),
        bn_b1=pc['bn1_b'].reshape(-1, 1).astype(np.float32),
        bn_g2=pc['bn2_g'].reshape(-1, 1).astype(np.float32),
        bn_b2=pc['bn2_b'].reshape(-1, 1).astype(np.float32),
    )

    def order_idx(ix):
        # (b, 3) -> flat order i = (g*3 + j)*128 + p for sample s = g*128 + p
        bb = ix.shape[0]
        return ix.reshape(bb // 128, 128, 3).transpose(0, 2, 1).reshape(-1)

    in_maps = []
    for c in range(n_cores):
        lo, hi = c * b, (c + 1) * b
        m = dict(shared)
        m['eidx'] = wrap_idx16(order_idx(efull[lo:hi]))
        m['fidx'] = wrap_idx16(order_idx(friends[lo:hi]))
        m['cidx'] = wrap_idx16(order_idx(cfull[lo:hi]))
        m['midx'] = map_idx[lo:hi, 0].astype(np.int32).reshape(1, b)
        in_maps.append(m)
    return in_maps


def kernel(**inputs):
    from concourse.bass_utils import run_bass_kernel_spmd
    b = B_FULL // NCORES
    pc = host_precompute(inputs)
    key = (b, NCORES)
    if key not in _NC_CACHE:
        _NC_CACHE[key] = build_nc(b, NCORES, with_collective=True)
    nc = _NC_CACHE[key]
    in_maps = make_core_inputs(inputs, pc, b, NCORES)
    res = run_bass_kernel_spmd(nc, in_maps, core_ids=list(range(NCORES)))
    out = np.concatenate([r['out'] for r in res.results], 0)
    return out



# revision 2
# speedup vs baseline: 1.6145x; 1.6145x over previous
"""Trainium2 Bass kernel for nn_EnhancedBrawlerPredictionModel (B=65536).

Data-parallel over 8 NeuronCores (8192 samples/core). Host folds all params
and performs the per-sample table lookups (pure index gathers, no per-sample
arithmetic); the device streams the gathered rows contiguously:
  - per-token q/k/v rows for enemy/friend self-attention (pos-emb and
    in_proj folded; v-bias and every purely additive constant is absorbed by
    the training-mode BatchNorm downstream),
  - cross-attention in_proj folded with fa/ea out_projs (32x32 mats),
  - fc1 folded per source block; map branch via one-hot matmul against a
    128x128 lhsT table,
  - counter influence via pre-masked/scaled row table
    ctab[valid*512+e] = (e!=0)*counter[e]/max(valid,1), rows gathered on
    host, accumulated into the fc3 PSUM with identity matmuls,
  - exact full-batch BN stats via two tiny AllReduces (sum, sum-of-squares).
"""

import numpy as np

import concourse.bass as bass
import concourse.bacc as bacc
import concourse.tile as tile
import concourse.mybir as mybir
from concourse.masks import make_identity

F32 = mybir.dt.float32
BF16 = mybir.dt.bfloat16
I32 = mybir.dt.int32

B_FULL = 65536
NCORES = 8
E, NH, DH, S = 32, 4, 8, 3
NB, NM, H = 512, 128, 128
AOP = mybir.AluOpType
AFT = mybir.ActivationFunctionType

# ---------------------------------------------------------------------------
# host-side precompute
# ---------------------------------------------------------------------------


def host_precompute(inp):
    f32 = np.float32
    emb = np.asarray(inp['brawler_emb'], f32)
    pos_w = np.asarray(inp['pos_w'], f32)
    pos_b = np.asarray(inp['pos_b'], f32)
    pos_emb = np.arange(S, dtype=f32)[:, None] * pos_w[None, :, 0] + pos_b

    def split_in(w, b):
        w = np.asarray(w, f32)
        b = np.asarray(b, f32)
        return (w[:E], w[E:2 * E], w[2 * E:], b[:E], b[E:2 * E], b[2 * E:])

    Wq_ea, Wk_ea, Wv_ea, bq_ea, bk_ea, bv_ea = split_in(inp['ea_in_w'], inp['ea_in_b'])
    Wq_fa, Wk_fa, Wv_fa, bq_fa, bk_fa, bv_fa = split_in(inp['fa_in_w'], inp['fa_in_b'])
    Wq_ca, Wk_ca, Wv_ca, bq_ca, bk_ca, bv_ca = split_in(inp['ca_in_w'], inp['ca_in_b'])
    Wout_ea, bout_ea = np.asarray(inp['ea_out_w'], f32), np.asarray(inp['ea_out_b'], f32)
    Wout_fa, bout_fa = np.asarray(inp['fa_out_w'], f32), np.asarray(inp['fa_out_b'], f32)
    Wout_ca, bout_ca = np.asarray(inp['ca_out_w'], f32), np.asarray(inp['ca_out_b'], f32)

    t_ea = emb[None, :, :] + pos_emb[:, None, :]
    # (3, 512, 96): per-token [q|k|v] rows
    ea_tab = np.concatenate([t_ea @ Wq_ea.T + bq_ea,
                             t_ea @ Wk_ea.T + bk_ea,
                             t_ea @ Wv_ea.T], -1)
    fa_tab = np.concatenate([emb @ Wq_fa.T + bq_fa,
                             emb @ Wk_fa.T + bk_fa,
                             emb @ Wv_fa.T], -1)

    Mq = Wq_ca @ Wout_fa
    bq_f = Mq @ bv_fa + Wq_ca @ bout_fa + bq_ca
    Mk = Wk_ca @ Wout_ea
    bk_f = Mk @ bv_ea + Wk_ca @ bout_ea + bk_ca
    Mv = Wv_ca @ Wout_ea

    fc1_w = np.asarray(inp['fc1_w'], f32)
    A_ca = fc1_w[:, 0:96].reshape(H, 3, E)
    A_ea = fc1_w[:, 96:192].reshape(H, 3, E)
    A_m = fc1_w[:, 192:224]
    CT = np.stack([(A_ca[:, i] @ Wout_ca).T for i in range(3)])   # (3, 32, 128)
    BT = np.stack([(A_ea[:, i] @ Wout_ea).T for i in range(3)])
    m_tab = np.asarray(inp['map_emb'], f32) @ A_m.T               # (128, 128)

    counter = np.asarray(inp['counter_matrix'], f32)
    nz = (np.arange(NB) != 0).astype(f32)[:, None]
    ctab = np.concatenate([nz * counter / max(v, 1) for v in range(4)], 0)

    W3aug = np.concatenate([np.asarray(inp['fc3_w'], f32).T,
                            np.asarray(inp['fc3_b'], f32)[None, :]], 0)

    cb = np.zeros((3, 32, 2, 128), f32)
    for i in range(3):
        cb[i, :, 0] = CT[i]
        cb[i, :, 1] = BT[i]
    return dict(
        ea_tab=ea_tab, fa_tab=fa_tab, ctab=ctab, m_tab=m_tab,
        # (96, 3, 32): [32i+k, which(q/k/v), out] - M.T replicated per token base
        mqkvT=np.tile(np.stack([Mq.T, Mk.T, Mv.T], 1), (3, 1, 1)),
        # (96, 2): per-partition bias columns for q/k (tiled over 3 tokens)
        bqk=np.stack([np.tile(bq_f, 3), np.tile(bk_f, 3)], 1),
        # (96, 2, 128): [32i+k, which(C/B), f1]
        cbT=cb.reshape(96, 2, 128),
        w2T=np.asarray(inp['fc2_w'], f32).T,
        w3aug=W3aug,
        bn1_g=np.asarray(inp['bn1_g'], f32), bn1_b=np.asarray(inp['bn1_b'], f32),
        bn2_g=np.asarray(inp['bn2_g'], f32), bn2_b=np.asarray(inp['bn2_b'], f32),
    )


# ---------------------------------------------------------------------------
# device kernel
# ---------------------------------------------------------------------------


def _attn(nc, pool, x, G, layout, out_ao):
    """Batch-major 3-token 4-head attention.
    layout 'A': x (128, G, 3, 96) token-major rows [q|k|v] (gathered tables).
    layout 'B': x (128, G, 288) = [q(3,32) | k(3,32) | v(3,32)].
    out_ao: (128, G, 3, 32) bf16, attention output pre-out_proj (v-bias-free).
    """
    if layout == 'A':
        qa = x[:, :, :, 0:32]

        def k_b(j):
            return x[:, :, j:j + 1, 32:64].to_broadcast([128, G, 3, 32])

        def v_i(j, i):
            return x[:, :, j, 64:96].rearrange("p g (h d) -> p g h d", d=DH)
    else:
        qa = x[:, :, 0:96].rearrange("p g (i d) -> p g i d", d=32)

        def k_b(j):
            return x[:, :, 96 + j * 32:96 + (j + 1) * 32].unsqueeze(2).to_broadcast(
                [128, G, 3, 32])

        def v_i(j, i):
            return x[:, :, 192 + j * 32:192 + (j + 1) * 32].rearrange(
                "p g (h d) -> p g h d", d=DH)

    M = pool.tile([128, G, 3, 3, E], BF16, tag="at_m")        # (g, j, i, d32)
    for j in range(3):
        nc.vector.tensor_tensor(out=M[:, :, j], in0=qa, in1=k_b(j), op=AOP.mult)
    # head-sum over d=8 via a strided add tree; (j,i,h) merge to one dim of 36
    M4 = M.rearrange("p g j i (h d) -> p g (j i h) d", d=DH)  # (128,G,36,8)
    t1 = pool.tile([128, G, 36, 4], BF16, tag="at_t1")
    nc.vector.tensor_tensor(out=t1, in0=M4[:, :, :, 0:4], in1=M4[:, :, :, 4:8],
                            op=AOP.add)
    t2 = pool.tile([128, G, 36, 2], BF16, tag="at_t2")
    nc.vector.tensor_tensor(out=t2, in0=t1[:, :, :, 0:2], in1=t1[:, :, :, 2:4],
                            op=AOP.add)
    s = pool.tile([128, G, 3, 3, NH], F32, tag="at_s")        # (j, i, h)
    nc.vector.tensor_tensor(out=s.rearrange("p g j i h -> p g (j i h)"),
                            in0=t2[:, :, :, 0], in1=t2[:, :, :, 1], op=AOP.add)
    e = pool.tile([128, G, 3, 3, NH], F32, tag="at_e")
    nc.scalar.activation(out=e, in_=s, func=AFT.Exp,
                         scale=float(1.0 / np.sqrt(DH)))
    den = pool.tile([128, G, 3, NH], F32, tag="at_den")       # (i, h)
    nc.vector.tensor_tensor(out=den, in0=e[:, :, 0], in1=e[:, :, 1], op=AOP.add)
    den2 = pool.tile([128, G, 3, NH], F32, tag="at_den2")
    nc.vector.tensor_tensor(out=den2, in0=den, in1=e[:, :, 2], op=AOP.add)
    r = pool.tile([128, G, 3, NH], F32, tag="at_r")
    rs = pool.tile([128, G, 3, NH], F32, tag="at_rs")
    nc.vector.reciprocal_approx_accurate(
        out=r.rearrange("p g i h -> p (g i h)"),
        in_=den2.rearrange("p g i h -> p (g i h)"),
        scratch=rs.rearrange("p g i h -> p (g i h)"))
    a = pool.tile([128, G, 3, 3, NH], BF16, tag="at_a")       # (j, i, h)
    nc.vector.tensor_tensor(
        out=a, in0=e, in1=r.unsqueeze(2).to_broadcast([128, G, 3, 3, NH]),
        op=AOP.mult)
    # AV: ao[i,h,d] = sum_j a[j,i,h] * v[j,h,d]; per (j,i): (G, 4, 8) ops.
    # No in-place accumulation (out must not alias an input on HW).
    ao_h = out_ao.rearrange("p g i (h d) -> p g i h d", d=DH)
    av0 = pool.tile([128, G, NH, DH], BF16, tag="at_av0")
    av1 = pool.tile([128, G, NH, DH], BF16, tag="at_av1")
    av2 = pool.tile([128, G, NH, DH], BF16, tag="at_av2")
    for i in range(3):
        for j, dst in ((0, av0), (1, av1), (2, av2)):
            a_b = a[:, :, j, i].unsqueeze(3).to_broadcast([128, G, NH, DH])
            nc.vector.tensor_tensor(out=dst[...], in0=a_b, in1=v_i(j, i),
                                    op=AOP.mult)
        s01 = pool.tile([128, G, NH, DH], BF16, tag="at_s01")
        nc.vector.tensor_tensor(out=s01[...], in0=av0[...], in1=av1[...],
                                op=AOP.add)
        nc.vector.tensor_tensor(out=ao_h[:, :, i], in0=s01[...], in1=av2[...],
                                op=AOP.add)


def build_nc(b, n_cores, with_collective=True, stage=5):
    assert b % 512 == 0
    nc = bacc.Bacc("TRN2", target_bir_lowering=False, debug=False,
                   num_devices=n_cores)

    G1 = min(8, b // 128)      # P1 chunk = 1024 samples
    CH1 = G1 * 128
    NCH1 = b // CH1
    G3 = min(4, b // 128)      # P3 chunk = 512 samples
    CH3 = G3 * 128
    NCH3 = b // CH3
    btot = float(b * (n_cores if with_collective else 1))

    dt_i = nc.dram_tensor
    # host-gathered per-sample rows (layout [p, group, token, width])
    xe_in = dt_i("xe", (128, (b // 128) * 3 * 96), BF16, kind="ExternalInput")
    xf_in = dt_i("xf", (128, (b // 128) * 3 * 96), BF16, kind="ExternalInput")
    ct_in = dt_i("ct", (128, (b // 128) * 3 * NB), BF16, kind="ExternalInput")
    midx = dt_i("midx", (1, b), I32, kind="ExternalInput")
    m_tab = dt_i("m_tab", (NM, 128), BF16, kind="ExternalInput")
    mqkvT = dt_i("mqkvT", (96, 3, 32), BF16, kind="ExternalInput")
    bqk = dt_i("bqk", (96, 2), F32, kind="ExternalInput")
    cbT = dt_i("cbT", (96, 2, 128), BF16, kind="ExternalInput")
    w2T = dt_i("w2T", (128, 64), BF16, kind="ExternalInput")
    w3aug = dt_i("w3aug", (65, NB), BF16, kind="ExternalInput")
    bn_g1 = dt_i("bn_g1", (H, 1), F32, kind="ExternalInput")
    bn_b1 = dt_i("bn_b1", (H, 1), F32, kind="ExternalInput")
    bn_g2 = dt_i("bn_g2", (64, 1), F32, kind="ExternalInput")
    bn_b2 = dt_i("bn_b2", (64, 1), F32, kind="ExternalInput")
    out_t = dt_i("out", (b, NB), BF16, kind="ExternalOutput")

    import contextlib
    with tile.TileContext(nc) as tc, contextlib.ExitStack() as ctx:
        singles = ctx.enter_context(tc.tile_pool(name="singles", bufs=1))
        dram = ctx.enter_context(tc.tile_pool(name="dram", bufs=1, space="DRAM"))

        # --- constants -----------------------------------------------------
        ident = singles.tile([128, 128], BF16)
        make_identity(nc, ident[:, :])

        def load(name, shape, dtype, src):
            t = singles.tile(shape, dtype, tag="c_" + name)
            nc.sync.dma_start(out=t[...], in_=src)
            return t

        c_mqkvT = load("mqkvT", [96, 3, 32], BF16, mqkvT[:, :, :])
        c_bqk = load("bqk", [96, 2], F32, bqk[:, :])
        c_cbT = load("cbT", [96, 2, 128], BF16, cbT[:, :, :])
        c_mtab = load("mtab", [NM, 128], BF16, m_tab[:, :])
        c_w2T = load("w2T", [128, 64], BF16, w2T[:, :])
        c_w3aug = load("w3aug", [65, NB], BF16, w3aug[:, :])
        c_g1 = load("g1", [H, 1], F32, bn_g1[:, :])
        c_b1 = load("b1", [H, 1], F32, bn_b1[:, :])
        c_g2 = load("g2", [64, 1], F32, bn_g2[:, :])
        c_b2 = load("b2", [64, 1], F32, bn_b2[:, :])
        iota_c = singles.tile([128, 1], I32)
        nc.gpsimd.iota(iota_c[:, :], pattern=[[0, 1]], base=0, channel_multiplier=1)

        h1 = singles.tile([128, b], BF16)
        a1 = singles.tile([128, b], BF16)
        h2 = singles.tile([64, b], BF16)
        a2aug = singles.tile([65, b], BF16)
        nc.vector.memset(a2aug[64:65, :], 1.0)

        # --- P1: attention chain + h1 --------------------------------------
        with tc.tile_pool(name="attn", bufs=2) as atp, \
             tc.tile_pool(name="gath", bufs=3) as gath, \
             tc.tile_pool(name="ao", bufs=2) as aopool, \
             tc.tile_pool(name="stag", bufs=2) as stag, \
             tc.tile_pool(name="mp", bufs=2) as mpool, \
             tc.tile_pool(name="ps_t", bufs=1, space="PSUM") as ps_t, \
             tc.tile_pool(name="ps_proj", bufs=2, space="PSUM") as ps_proj, \
             tc.tile_pool(name="ps_xc", bufs=1, space="PSUM") as ps_xc, \
             tc.tile_pool(name="ps_h1", bufs=1, space="PSUM") as ps_h1:
            for ch in range(NCH1):
                xe = gath.tile([128, G1, 3, 96], BF16, tag="xe")
                xf = gath.tile([128, G1, 3, 96], BF16, tag="xf")
                c0 = ch * G1 * 3 * 96
                c1 = c0 + G1 * 3 * 96
                nc.sync.dma_start(out=xe[...], in_=xe_in[:, c0:c1])
                nc.sync.dma_start(out=xf[...], in_=xf_in[:, c0:c1])

                ao_e = aopool.tile([128, G1, 3, 32], BF16, tag="ao_e")
                ao_f = aopool.tile([128, G1, 3, 32], BF16, tag="ao_f")
                _attn(nc, atp, xe, G1, 'A', ao_e)
                _attn(nc, atp, xf, G1, 'A', ao_f)

                for sc in range(G1 // 4):          # 512-sample sub-chunks
                    g0 = sc * 4
                    aofT_ps = ps_t.tile([96, 512], BF16, tag="aofT")
                    aoeT_ps = ps_t.tile([96, 512], BF16, tag="aoeT")
                    for t in range(4):
                        nc.tensor.transpose(
                            aofT_ps[:, t * 128:(t + 1) * 128],
                            ao_f[:, g0 + t].rearrange("p i d -> p (i d)"),
                            ident[:, :])
                        nc.tensor.transpose(
                            aoeT_ps[:, t * 128:(t + 1) * 128],
                            ao_e[:, g0 + t].rearrange("p i d -> p (i d)"),
                            ident[:, :])
                    aofT = stag.tile([96, 512], BF16, tag="aofT_s")
                    aoeT = stag.tile([96, 512], BF16, tag="aoeT_s")
                    nc.scalar.activation(out=aofT[...], in_=aofT_ps[...], func=AFT.Copy)
                    nc.scalar.activation(out=aoeT[...], in_=aoeT_ps[...], func=AFT.Copy)
                    col0 = (ch * (G1 // 4) + sc) * 512

                    # ca projections, feature-major
                    qkvT = stag.tile([96, 3, 512], BF16, tag="qkvT_s")
                    for w in range(3):
                        src = aofT if w == 0 else aoeT
                        pw = ps_proj.tile([96, 512], F32, tag="projT")
                        for i in range(3):
                            sl = slice(i * 32, (i + 1) * 32)
                            nc.tensor.matmul(pw[sl, :], c_mqkvT[sl, w, :], src[sl, :],
                                             start=True, stop=True,
                                             tile_position=(32 * i, 32 * i))
                        if w < 2:
                            nc.vector.tensor_scalar(
                                out=qkvT[:, w], in0=pw[...],
                                scalar1=c_bqk[:, w:w + 1], scalar2=None,
                                op0=AOP.add)
                        else:
                            nc.scalar.activation(out=qkvT[:, w], in_=pw[...],
                                                 func=AFT.Copy)

                    # back to batch-major: per group [q(3,32)|k(3,32)|v(3,32)],
                    # groups padded to 512 elems for psum bank alignment
                    xc_ps = ps_xc.tile([128, 4, 512], BF16, tag="xc_ps")
                    for t in range(4):
                        for w in range(3):
                            nc.tensor.transpose(
                                xc_ps[:, t, w * 96:(w + 1) * 96],
                                qkvT[:, w, t * 128:(t + 1) * 128],
                                ident[0:96, 0:96])
                    xc = mpool.tile([128, 4, 288], BF16, tag="xc")
                    nc.scalar.activation(out=xc[...], in_=xc_ps[:, :, 0:288],
                                         func=AFT.Copy)

                    att_c = aopool.tile([128, 4, 3, 32], BF16, tag="att_c")
                    _attn(nc, atp, xc, 4, 'B', att_c)

                    actT_ps = ps_t.tile([96, 512], BF16, tag="actT")
                    for t in range(4):
                        nc.tensor.transpose(
                            actT_ps[:, t * 128:(t + 1) * 128],
                            att_c[:, t].rearrange("p i d -> p (i d)"),
                            ident[:, :])
                    actT = stag.tile([96, 512], BF16, tag="actT_s")
                    nc.scalar.activation(out=actT[...], in_=actT_ps[...], func=AFT.Copy)

                    # map one-hot for this 512-chunk
                    mrep = mpool.tile([128, 512], I32, tag="mrep")
                    nc.sync.dma_start(
                        out=mrep[...],
                        in_=midx[0:1, col0:col0 + 512].to_broadcast([128, 512]))
                    oh = mpool.tile([128, 512], BF16, tag="oh")
                    nc.vector.tensor_tensor(
                        out=oh[...], in0=mrep[...],
                        in1=iota_c[:, 0:1].to_broadcast([128, 512]), op=AOP.is_equal)

                    # h1 += sum_i C_i.T@att_ca_i + sum_i B_i.T@ao_e_i + m_tab@oh.
                    # The per-token sums fold into single K=96 matmuls (cbT rows
                    # are [C0.T;C1.T;C2.T] / [B0.T;B1.T;B2.T]).
                    h1_ps = ps_h1.tile([128, 512], F32, tag="h1ps")
                    nc.tensor.matmul(h1_ps[...], c_cbT[:, 0, :], actT[...],
                                     start=True, stop=False)
                    nc.tensor.matmul(h1_ps[...], c_cbT[:, 1, :], aoeT[...],
                                     start=False, stop=False)
                    nc.tensor.matmul(h1_ps[...], c_mtab[:, :], oh[...],
                                     start=False, stop=True)
                    nc.scalar.activation(out=h1[:, col0:col0 + 512], in_=h1_ps[...],
                                         func=AFT.Copy)

        # --- BN (exact global stats) ---------------------------------------
        def bn_stats_apply(src, parts, g_col, b_col, relu_out, cc_name):
            s1 = singles.tile([parts, 1], F32, tag=cc_name + "_s1")
            nc.vector.tensor_reduce(out=s1[...], in_=src, axis=mybir.AxisListType.X,
                                    op=AOP.add)
            q1 = singles.tile([parts, 1], F32, tag=cc_name + "_q1")
            nc.scalar.activation(out=relu_out, in_=src, func=AFT.Square,
                                 accum_out=q1[...])
            if with_collective:
                cc_in = dram.tile([parts, 2], F32, tag=cc_name + "_in")
                cc_out = nc.dram_tensor(cc_name + "_out", (parts, 2), F32,
                                        kind="Internal", addr_space="Shared")
                nc.sync.dma_start(out=cc_in[:, 0:1], in_=s1[...])
                nc.sync.dma_start(out=cc_in[:, 1:2], in_=q1[...])
                nc.gpsimd.collective_compute(
                    "AllReduce", AOP.add,
                    replica_groups=[list(range(n_cores))],
                    ins=[cc_in[:, :].opt()], outs=[cc_out[:, :].opt()])
                sq = singles.tile([parts, 2], F32, tag=cc_name + "_sq")
                nc.sync.dma_start(out=sq[...], in_=cc_out[:, :])
                s1g, q1g = sq[:, 0:1], sq[:, 1:2]
            else:
                s1g, q1g = s1[:, :], q1[:, :]
            mean = singles.tile([parts, 1], F32, tag=cc_name + "_mean")
            nc.vector.tensor_scalar_mul(mean[...], s1g, 1.0 / btot)
            msq = singles.tile([parts, 1], F32, tag=cc_name + "_msq")
            nc.vector.tensor_scalar_mul(msq[...], q1g, 1.0 / btot)
            m2 = singles.tile([parts, 1], F32, tag=cc_name + "_m2")
            nc.vector.tensor_tensor(out=m2[...], in0=mean[...], in1=mean[...],
                                    op=AOP.mult)
            var = singles.tile([parts, 1], F32, tag=cc_name + "_var")
            nc.vector.tensor_tensor(out=var[...], in0=msq[...], in1=m2[...],
                                    op=AOP.subtract)
            eps = singles.tile([parts, 1], F32, tag=cc_name + "_eps")
            nc.vector.memset(eps[...], 1e-5)
            std = singles.tile([parts, 1], F32, tag=cc_name + "_std")
            nc.scalar.activation(out=std[...], in_=var[...], func=AFT.Sqrt,
                                 bias=eps[...])
            rstd = singles.tile([parts, 1], F32, tag=cc_name + "_rstd")
            nc.vector.reciprocal(out=rstd[...], in_=std[...])
            scale = singles.tile([parts, 1], F32, tag=cc_name + "_scale")
            nc.vector.tensor_tensor(out=scale[...], in0=g_col[...], in1=rstd[...],
                                    op=AOP.mult)
            mscale = singles.tile([parts, 1], F32, tag=cc_name + "_ms")
            nc.vector.tensor_tensor(out=mscale[...], in0=mean[...], in1=scale[...],
                                    op=AOP.mult)
            nbias = singles.tile([parts, 1], F32, tag=cc_name + "_nb")
            nc.vector.tensor_tensor(out=nbias[...], in0=b_col[...], in1=mscale[...],
                                    op=AOP.subtract)
            nc.scalar.activation(out=relu_out, in_=src, func=AFT.Relu,
                                 bias=nbias[...], scale=scale[...])

        bn_stats_apply(h1[:, :], 128, c_g1, c_b1, a1[:, :], "cc1")

        with tc.tile_pool(name="ps_h2", bufs=2, space="PSUM") as ps_h2:
            for sc in range(b // 512):
                h2_ps = ps_h2.tile([64, 512], F32, tag="h2ps")
                nc.tensor.matmul(h2_ps[...], c_w2T[:, :],
                                 a1[:, sc * 512:(sc + 1) * 512],
                                 start=True, stop=True)
                nc.scalar.activation(out=h2[:, sc * 512:(sc + 1) * 512],
                                     in_=h2_ps[...], func=AFT.Copy)

        bn_stats_apply(h2[:, :], 64, c_g2, c_b2, a2aug[0:64, :], "cc2")

        # --- P3: fc3 + counter + writeback ---------------------------------
        with tc.tile_pool(name="ps_o", bufs=4, space="PSUM") as ps_o, \
             tc.tile_pool(name="ct", bufs=3) as ctpool, \
             tc.tile_pool(name="ost", bufs=2) as ost:
            out_r = out_t[:, :].rearrange("(g p) n -> p g n", p=128)
            for ch in range(NCH3):
                ct = ctpool.tile([128, G3, 3, NB], BF16, tag="ct")
                c0 = ch * G3 * 3 * NB
                c1 = c0 + G3 * 3 * NB
                nc.sync.dma_start(out=ct[...], in_=ct_in[:, c0:c1])
                ostg = ost.tile([128, G3, NB], BF16, tag="ostg")
                for g in range(G3):
                    col0 = ch * CH3 + g * 128
                    o_ps = ps_o.tile([128, NB], F32, tag="ops")
                    nc.tensor.matmul(o_ps[...], a2aug[:, col0:col0 + 128],
                                     c_w3aug[:, :], start=True, stop=False)
                    for j in range(3):
                        nc.tensor.matmul(o_ps[...], ident[:, :], ct[:, g, j, :],
                                         start=False, stop=(j == 2))
                    nc.scalar.activation(out=ostg[:, g], in_=o_ps[...], func=AFT.Copy)
                nc.sync.dma_start(out=out_r[:, ch * G3:(ch + 1) * G3, :],
                                  in_=ostg[...])

    nc.compile()
    return nc


# ---------------------------------------------------------------------------
# host wrapper
# ---------------------------------------------------------------------------

_NC_CACHE = {}


def make_core_inputs(inputs, pc, b, n_cores):
    import ml_dtypes
    bf16 = ml_dtypes.bfloat16
    friends = np.asarray(inputs['friends'], np.int64)
    enemies = np.asarray(inputs['enemies'], np.int64)
    map_idx = np.asarray(inputs['map_idx'], np.int64)

    valid = (enemies != 0).sum(1)
    cfull = valid[:, None] * NB + enemies

    # bf16 tables so host gathers are raw row copies (no per-row conversion)
    ea_tab = pc['ea_tab'].astype(bf16)          # (3, 512, 96)
    fa_tab = pc['fa_tab'].astype(bf16)          # (512, 96)
    ctab = pc['ctab'].astype(bf16)              # (2048, 512)

    shared = dict(
        m_tab=pc['m_tab'].astype(bf16),
        mqkvT=pc['mqkvT'].astype(bf16), bqk=pc['bqk'].astype(np.float32),
        cbT=pc['cbT'].astype(bf16), w2T=pc['w2T'].astype(bf16),
        w3aug=pc['w3aug'].astype(bf16),
        bn_g1=pc['bn1_g'].reshape(-1, 1).astype(np.float32),
        bn_b1=pc['bn1_b'].reshape(-1, 1).astype(np.float32),
        bn_g2=pc['bn2_g'].reshape(-1, 1).astype(np.float32),
        bn_b2=pc['bn2_b'].reshape(-1, 1).astype(np.float32),
    )

    tok3 = np.arange(3)[None, None, :]

    in_maps = []
    for c in range(n_cores):
        lo, hi = c * b, (c + 1) * b
        m = dict(shared)
        # sample s = g*128 + p -> host array [p, g, t, w] -> flat [128, -1]
        e = enemies[lo:hi].reshape(b // 128, 128, 3)
        f = friends[lo:hi].reshape(b // 128, 128, 3)
        cf = cfull[lo:hi].reshape(b // 128, 128, 3)
        m['xe'] = np.ascontiguousarray(
            ea_tab[tok3, e].transpose(1, 0, 2, 3)).reshape(128, -1)
        m['xf'] = np.ascontiguousarray(
            fa_tab[f].transpose(1, 0, 2, 3)).reshape(128, -1)
        m['ct'] = np.ascontiguousarray(
            ctab[cf].transpose(1, 0, 2, 3)).reshape(128, -1)
        m['midx'] = map_idx[lo:hi, 0].astype(np.int32).reshape(1, b)
        in_maps.append(m)
    return in_maps


def kernel(**inputs):
    from concourse.bass_utils import run_bass_kernel_spmd
    b = B_FULL // NCORES
    pc = host_precompute(inputs)
    key = (b, NCORES)
    if key not in _NC_CACHE:
        _NC_CACHE[key] = build_nc(b, NCORES, with_collective=True)
    nc = _NC_CACHE[key]
    in_maps = make_core_inputs(inputs, pc, b, NCORES)
    res = run_bass_kernel_spmd(nc, in_maps, core_ids=list(range(NCORES)))
    out = np.concatenate([np.asarray(r['out'], np.float32)
                          for r in res.results], 0)
    return out


# revision 6
# speedup vs baseline: 2.2358x; 1.3848x over previous
"""Trainium2 Bass kernel for nn_EnhancedBrawlerPredictionModel (B=65536).

Data-parallel over 8 NeuronCores (8192 samples/core). All parameter algebra is
folded on the host into input-independent lookup tables; per-sample HOST work
is index-gathers only (no per-sample arithmetic). The device computes the
three softmaxes, the attention-weighted sums, the cross-attention bilinear
score contraction, fc1/fc2/fc3, exact-global-batch BatchNorm (two tiny
AllReduces), and the counter-matrix influence.

Table folding:
  - self-attention scores are bilinear in (token, embedding-id) pairs ->
    gathered per-sample from T_EA[(i,e_i),(j,e_j),h], T_FA[f_i,f_j,h].
  - cross-attention q/k are affine in the fa/ea attention *weights* ->
    s_ca = af~^T Tt ae~ with Tt a per-sample gathered 4x4x4 table
    (bias row/col augmented).
  - h1 = sum_i D_i z_i + sum_i B_i ao_e_i + m_tab[map] with
    z_i = sum_j a_ca[j,i] ao_e_j (Mv and fc1 blocks folded into D_i).
  - counter influence: unscaled pair-sum table P2[e1,e2] + single row,
    scaled by 1/valid on device via a diagonal matmul.
"""

import numpy as np

import concourse.bass as bass
import concourse.bacc as bacc
import concourse.tile as tile
import concourse.mybir as mybir
from concourse.masks import make_identity

F32 = mybir.dt.float32
BF16 = mybir.dt.bfloat16
I32 = mybir.dt.int32

B_FULL = 65536
NCORES = 8
E, NH, DH, S = 32, 4, 8, 3
NB, NM, H = 512, 128, 128
AOP = mybir.AluOpType
AFT = mybir.ActivationFunctionType

# per-sample record layout (bf16 elements)
OFF_SEA, OFF_SFA, OFF_T, OFF_VE, OFF_VF, OFF_MR, REC = 0, 36, 72, 136, 232, 328, 456

# ---------------------------------------------------------------------------
# host-side precompute (input-independent tables)
# ---------------------------------------------------------------------------


def host_precompute(inp):
    f32 = np.float32
    emb = np.asarray(inp['brawler_emb'], f32)
    pos_w = np.asarray(inp['pos_w'], f32)
    pos_b = np.asarray(inp['pos_b'], f32)
    pos_emb = np.arange(S, dtype=f32)[:, None] * pos_w[None, :, 0] + pos_b

    def split_in(w, b):
        w = np.asarray(w, f32)
        b = np.asarray(b, f32)
        return (w[:E], w[E:2 * E], w[2 * E:], b[:E], b[E:2 * E], b[2 * E:])

    Wq_ea, Wk_ea, Wv_ea, bq_ea, bk_ea, bv_ea = split_in(inp['ea_in_w'], inp['ea_in_b'])
    Wq_fa, Wk_fa, Wv_fa, bq_fa, bk_fa, bv_fa = split_in(inp['fa_in_w'], inp['fa_in_b'])
    Wq_ca, Wk_ca, Wv_ca, bq_ca, bk_ca, bv_ca = split_in(inp['ca_in_w'], inp['ca_in_b'])
    Wout_ea = np.asarray(inp['ea_out_w'], f32)
    bout_ea = np.asarray(inp['ea_out_b'], f32)
    Wout_fa = np.asarray(inp['fa_out_w'], f32)
    bout_fa = np.asarray(inp['fa_out_b'], f32)
    Wout_ca = np.asarray(inp['ca_out_w'], f32)

    t_ea = emb[None] + pos_emb[:, None]                  # (3, 512, 32)

    def hdot(A, B):
        return np.einsum('...hd,...hd->...h',
                         A.reshape(*A.shape[:-1], NH, DH),
                         B.reshape(*B.shape[:-1], NH, DH))

    q_ea = t_ea @ Wq_ea.T + bq_ea
    k_ea = t_ea @ Wk_ea.T + bk_ea
    T_EA = hdot(q_ea[:, :, None, None], k_ea[None, None])    # (3,512,3,512,4)
    q_fa = emb @ Wq_fa.T + bq_fa
    k_fa = emb @ Wk_fa.T + bk_fa
    T_FA = hdot(q_fa[:, None], k_fa[None])                   # (512, 512, 4)

    v_ea = t_ea @ Wv_ea.T                                    # (3, 512, 32)
    v_fa = emb @ Wv_fa.T                                     # (512, 32)

    Mq = Wq_ca @ Wout_fa
    bq_f = Mq @ bv_fa + Wq_ca @ bout_fa + bq_ca
    Mk = Wk_ca @ Wout_ea
    bk_f = Mk @ bv_ea + Wk_ca @ bout_ea + bk_ca
    Mv = Wv_ca @ Wout_ea

    TqA = np.concatenate([v_fa @ Mq.T, bq_f[None]], 0)       # (513, 32)
    TkA = np.concatenate([(v_ea @ Mk.T).reshape(3 * NB, E), bk_f[None]], 0)
    TCA = hdot(TqA[:, None], TkA[None])                      # (513, 1537, 4)

    fc1_w = np.asarray(inp['fc1_w'], f32)
    A_ca = fc1_w[:, 0:96].reshape(H, 3, E)
    A_ea = fc1_w[:, 96:192].reshape(H, 3, E)
    A_m = fc1_w[:, 192:224]
    m_tab = np.asarray(inp['map_emb'], f32) @ A_m.T          # (128, 128)

    # db[i,:,0] = D_i.T (Mv folded), db[i,:,1] = B_i.T
    db = np.zeros((3, 32, 2, 128), f32)
    for i in range(3):
        db[i, :, 0] = ((A_ca[:, i] @ Wout_ca) @ Mv).T
        db[i, :, 1] = (A_ea[:, i] @ Wout_ea).T

    counter = np.asarray(inp['counter_matrix'], f32)
    nz = (np.arange(NB) != 0).astype(f32)
    cn = nz[:, None] * counter                               # (512, 512)

    W3aug = np.concatenate([np.asarray(inp['fc3_w'], f32).T,
                            np.asarray(inp['fc3_b'], f32)[None, :]], 0)

    return dict(
        T_EA=T_EA, T_FA=T_FA, TCA=TCA, v_ea=v_ea, v_fa=v_fa, m_tab=m_tab,
        cn=cn, dbT=db.reshape(96, 2, 128), w2T=np.asarray(inp['fc2_w'], f32).T,
        w3aug=W3aug,
        bn1_g=np.asarray(inp['bn1_g'], f32), bn1_b=np.asarray(inp['bn1_b'], f32),
        bn2_g=np.asarray(inp['bn2_g'], f32), bn2_b=np.asarray(inp['bn2_b'], f32),
    )


# ---------------------------------------------------------------------------
# device kernel
# ---------------------------------------------------------------------------


def build_nc(b, n_cores, with_collective=True, stage=5):
    assert b % 1024 == 0
    nc = bacc.Bacc("TRN2", target_bir_lowering=False, debug=False,
                   num_devices=n_cores)

    G1 = 8                     # P1 chunk = 1024 samples
    CH1 = G1 * 128
    NCH1 = b // CH1
    G3 = 8                     # P3 chunk = 1024 samples
    CH3 = G3 * 128
    NCH3 = b // CH3
    NGG = b // 128             # total 128-sample groups
    btot = float(b * (n_cores if with_collective else 1))
    rsq = float(1.0 / np.sqrt(DH))

    dt_i = nc.dram_tensor
    xrec = dt_i("xrec", (128, NGG * REC), BF16, kind="ExternalInput")
    ct_in = dt_i("ct", (128, NGG * 2 * NB), BF16, kind="ExternalInput")
    rv_in = dt_i("rv", (128, NGG), F32, kind="ExternalInput")
    dbT = dt_i("dbT", (96, 2, 128), BF16, kind="ExternalInput")
    w2T = dt_i("w2T", (128, 64), BF16, kind="ExternalInput")
    w3aug = dt_i("w3aug", (65, NB), BF16, kind="ExternalInput")
    bn_g1 = dt_i("bn_g1", (H, 1), F32, kind="ExternalInput")
    bn_b1 = dt_i("bn_b1", (H, 1), F32, kind="ExternalInput")
    bn_g2 = dt_i("bn_g2", (64, 1), F32, kind="ExternalInput")
    bn_b2 = dt_i("bn_b2", (64, 1), F32, kind="ExternalInput")
    out_t = dt_i("out", (b, NB), BF16, kind="ExternalOutput")

    import contextlib
    with tile.TileContext(nc) as tc, contextlib.ExitStack() as ctx:
        singles = ctx.enter_context(tc.tile_pool(name="singles", bufs=1))
        dram = ctx.enter_context(tc.tile_pool(name="dram", bufs=1, space="DRAM"))

        ident = singles.tile([128, 128], BF16)
        make_identity(nc, ident[:, :])

        def load(name, shape, dtype, src):
            t = singles.tile(shape, dtype, tag="c_" + name)
            nc.sync.dma_start(out=t[...], in_=src)
            return t

        c_dbT = load("dbT", [96, 2, 128], BF16, dbT[:, :, :])
        c_w2T = load("w2T", [128, 64], BF16, w2T[:, :])
        c_w3aug = load("w3aug", [65, NB], BF16, w3aug[:, :])
        c_g1 = load("g1", [H, 1], F32, bn_g1[:, :])
        c_b1 = load("b1", [H, 1], F32, bn_b1[:, :])
        c_g2 = load("g2", [64, 1], F32, bn_g2[:, :])
        c_b2 = load("b2", [64, 1], F32, bn_b2[:, :])
        c_rv = load("rv", [128, NGG], F32, rv_in[:, :])

        h1 = singles.tile([128, b], BF16)
        a1 = singles.tile([128, b], BF16)
        h2 = singles.tile([64, b], BF16)
        a2aug = singles.tile([65, b], BF16)
        nc.vector.memset(a2aug[64:65, :], 1.0)
        # per-512-chunk BN stat partials (sum / sum-of-squares)
        s1p = singles.tile([128, b // 512], F32)
        q1p = singles.tile([128, b // 512], F32)
        s2p = singles.tile([64, b // 512], F32)
        q2p = singles.tile([64, b // 512], F32)

        # --- P1: softmax / AV / ca-bilinear chain + h1 ---------------------
        with tc.tile_pool(name="attn", bufs=2) as atp, \
             tc.tile_pool(name="gath", bufs=3) as gath, \
             tc.tile_pool(name="ao", bufs=2) as aopool, \
             tc.tile_pool(name="stag", bufs=2) as stag, \
             tc.tile_pool(name="ps_t", bufs=1, space="PSUM") as ps_t, \
             tc.tile_pool(name="ps_h1", bufs=2, space="PSUM") as ps_h1:
            for ch in range(NCH1):
                xr = gath.tile([128, G1, REC], BF16, tag="xr")
                nc.sync.dma_start(out=xr[...],
                                  in_=xrec[:, ch * G1 * REC:(ch + 1) * G1 * REC])

                # ea+fa softmax from gathered scores (layout w, j(key), i, h)
                e_b = atp.tile([128, G1, 2, 3, 12], F32, tag="e_b")
                nc.scalar.activation(
                    out=e_b.rearrange("p g w j x -> p g (w j x)"),
                    in_=xr[:, :, 0:72], func=AFT.Exp, scale=rsq)
                den = atp.tile([128, G1, 2, 12], F32, tag="den")
                nc.vector.tensor_tensor(out=den, in0=e_b[:, :, :, 0],
                                        in1=e_b[:, :, :, 1], op=AOP.add)
                den2 = atp.tile([128, G1, 2, 12], F32, tag="den2")
                nc.vector.tensor_tensor(out=den2, in0=den, in1=e_b[:, :, :, 2],
                                        op=AOP.add)
                r_b = atp.tile([128, G1, 2, 12], F32, tag="r_b")
                rs = atp.tile([128, G1, 2, 12], F32, tag="rs")
                nc.vector.reciprocal_approx_accurate(
                    out=r_b.rearrange("p g w x -> p (g w x)"),
                    in_=den2.rearrange("p g w x -> p (g w x)"),
                    scratch=rs.rearrange("p g w x -> p (g w x)"))
                a_b = atp.tile([128, G1, 2, 3, 12], BF16, tag="a_b")
                for w in range(2):
                    nc.vector.tensor_tensor(
                        out=a_b[:, :, w], in0=e_b[:, :, w],
                        in1=r_b[:, :, w].unsqueeze(2)
                        .to_broadcast([128, G1, 3, 12]),
                        op=AOP.mult)

                # AV for ea and fa: ao[i] = sum_j a[j,i] (x) v[j]
                ao_e = aopool.tile([128, G1, 3, E], BF16, tag="ao_e")
                ao_f = aopool.tile([128, G1, 3, E], BF16, tag="ao_f")

                def attn_av(w, a_t, v_of_j, ao):
                    # v_of_j(j) -> [128, G1, NH, DH]-broadcastable AP
                    avs = []
                    for j in range(3):
                        av = atp.tile([128, G1, 3, NH, DH], BF16,
                                      tag=f"av{w}{j}")
                        v_in = v_of_j(j)
                        for i in range(3):
                            a_in = a_t[:, :, j, 4 * i:4 * (i + 1)] \
                                .unsqueeze(3).to_broadcast([128, G1, NH, DH])
                            nc.vector.tensor_tensor(out=av[:, :, i],
                                                    in0=a_in, in1=v_in,
                                                    op=AOP.mult)
                        avs.append(av)
                    s01 = atp.tile([128, G1, 3, NH, DH], BF16, tag=f"s01{w}")
                    nc.vector.tensor_tensor(out=s01[...], in0=avs[0][...],
                                            in1=avs[1][...], op=AOP.add)
                    nc.vector.tensor_tensor(
                        out=ao.rearrange("p g i (h d) -> p g i h d", d=DH),
                        in0=s01[...], in1=avs[2][...], op=AOP.add)

                attn_av(0, a_b[:, :, 0],
                        lambda j: xr[:, :, OFF_VE + E * j:OFF_VE + E * (j + 1)]
                        .rearrange("p g (h d) -> p g h d", d=DH), ao_e)
                attn_av(1, a_b[:, :, 1],
                        lambda j: xr[:, :, OFF_VF + E * j:OFF_VF + E * (j + 1)]
                        .rearrange("p g (h d) -> p g h d", d=DH), ao_f)

                # ca bilinear scores: s[j,i,h] = af~_i^T Tt ae~_j
                # U[i,m,h] = sum_l af[l,i,h] Tt[l,m,h]  (+ bias row l'=3)
                us = []
                for l in range(3):
                    u = atp.tile([128, G1, 3, 4, NH], BF16, tag=f"u{l}")
                    tl = xr[:, :, OFF_T + 16 * l:OFF_T + 16 * (l + 1)] \
                        .rearrange("p g (m h) -> p g m h", h=NH)
                    for i in range(3):
                        af = a_b[:, :, 1, l, 4 * i:4 * (i + 1)] \
                            .unsqueeze(2).to_broadcast([128, G1, 4, NH])
                        nc.vector.tensor_tensor(out=u[:, :, i], in0=af,
                                                in1=tl, op=AOP.mult)
                    us.append(u)
                t3b = xr[:, :, OFF_T + 48:OFF_T + 64].rearrange(
                    "p g (m h) -> p g m h", h=NH).unsqueeze(2) \
                    .to_broadcast([128, G1, 3, 4, NH])
                us1 = atp.tile([128, G1, 3, 4, NH], BF16, tag="us1")
                us2 = atp.tile([128, G1, 3, 4, NH], BF16, tag="us2")
                U = atp.tile([128, G1, 3, 4, NH], BF16, tag="U")
                nc.vector.tensor_tensor(out=us1[...], in0=us[0][...],
                                        in1=us[1][...], op=AOP.add)
                nc.vector.tensor_tensor(out=us2[...], in0=us[2][...], in1=t3b,
                                        op=AOP.add)
                nc.vector.tensor_tensor(out=U[...], in0=us1[...], in1=us2[...],
                                        op=AOP.add)

                # s_ca[j,i,h] = sum_m U[i,m,h] ae[m,j,h]  (+ bias col m'=3)
                sc = atp.tile([128, G1, 3, 3, NH], BF16, tag="sc")
                for i in range(3):
                    vs = []
                    for m in range(3):
                        v = atp.tile([128, G1, 3, NH], BF16, tag=f"vv{i}{m}")
                        ui = U[:, :, i, m, :].unsqueeze(2) \
                            .to_broadcast([128, G1, 3, NH])
                        ae = a_b[:, :, 0, m].rearrange(
                            "p g (j2 h) -> p g j2 h", h=NH)
                        nc.vector.tensor_tensor(out=v[...], in0=ui, in1=ae,
                                                op=AOP.mult)
                        vs.append(v)
                    u3 = U[:, :, i, 3, :].unsqueeze(2) \
                        .to_broadcast([128, G1, 3, NH])
                    vs1 = atp.tile([128, G1, 3, NH], BF16, tag=f"vs1{i}")
                    vs2 = atp.tile([128, G1, 3, NH], BF16, tag=f"vs2{i}")
                    nc.vector.tensor_tensor(out=vs1[...], in0=vs[0][...],
                                            in1=vs[1][...], op=AOP.add)
                    nc.vector.tensor_tensor(out=vs2[...], in0=vs[2][...],
                                            in1=u3, op=AOP.add)
                    nc.vector.tensor_tensor(out=sc[:, :, :, i, :],
                                            in0=vs1[...], in1=vs2[...],
                                            op=AOP.add)

                # ca softmax
                e_c = atp.tile([128, G1, 3, 12], F32, tag="e_c")
                nc.scalar.activation(
                    out=e_c.rearrange("p g j x -> p g (j x)"),
                    in_=sc.rearrange("p g j i h -> p g (j i h)"),
                    func=AFT.Exp, scale=rsq)
                dc = atp.tile([128, G1, 12], F32, tag="dc")
                nc.vector.tensor_tensor(out=dc, in0=e_c[:, :, 0],
                                        in1=e_c[:, :, 1], op=AOP.add)
                dc2 = atp.tile([128, G1, 12], F32, tag="dc2")
                nc.vector.tensor_tensor(out=dc2, in0=dc, in1=e_c[:, :, 2],
                                        op=AOP.add)
                r_c = atp.tile([128, G1, 12], F32, tag="r_c")
                rcs = atp.tile([128, G1, 12], F32, tag="rcs")
                nc.vector.reciprocal_approx_accurate(
                    out=r_c.rearrange("p g x -> p (g x)"),
                    in_=dc2.rearrange("p g x -> p (g x)"),
                    scratch=rcs.rearrange("p g x -> p (g x)"))
                a_c = atp.tile([128, G1, 3, 12], BF16, tag="a_c")
                nc.vector.tensor_tensor(
                    out=a_c, in0=e_c,
                    in1=r_c.unsqueeze(2).to_broadcast([128, G1, 3, 12]),
                    op=AOP.mult)

                # z AV over ao_e
                z = aopool.tile([128, G1, 3, E], BF16, tag="z")
                attn_av(2, a_c[:, :, :, :],
                        lambda j: ao_e[:, :, j].rearrange(
                            "p g (h d) -> p g h d", d=DH), z)

                # per 512-sample sub-chunk: transposes + h1 accumulation
                for sub in range(G1 // 4):
                    g0 = sub * 4
                    sidx = ch * (G1 // 4) + sub
                    col0 = sidx * 512
                    aoeT_ps = ps_t.tile([96, 512], BF16, tag="aoeT")
                    zT_ps = ps_t.tile([96, 512], BF16, tag="zT")
                    for t in range(4):
                        nc.tensor.transpose(
                            aoeT_ps[:, t * 128:(t + 1) * 128],
                            ao_e[:, g0 + t].rearrange("p i d -> p (i d)"),
                            ident[:, :])
                        nc.tensor.transpose(
                            zT_ps[:, t * 128:(t + 1) * 128],
                            z[:, g0 + t].rearrange("p i d -> p (i d)"),
                            ident[:, :])
                    aoeT = stag.tile([96, 512], BF16, tag="aoeT_s")
                    zT = stag.tile([96, 512], BF16, tag="zT_s")
                    nc.scalar.activation(out=aoeT[...], in_=aoeT_ps[...],
                                         func=AFT.Copy)
                    nc.scalar.activation(out=zT[...], in_=zT_ps[...],
                                         func=AFT.Copy)

                    h1_ps = ps_h1.tile([128, 512], F32, tag="h1ps")
                    nc.tensor.matmul(h1_ps[...], c_dbT[:, 0, :], zT[...],
                                     start=True, stop=False)
                    nc.tensor.matmul(h1_ps[...], c_dbT[:, 1, :], aoeT[...],
                                     start=False, stop=False)
                    for t in range(4):
                        nc.tensor.matmul(
                            h1_ps[:, t * 128:(t + 1) * 128],
                            xr[:, g0 + t, OFF_MR:OFF_MR + 128],
                            ident[:, :], start=False, stop=(t == 3))
                    nc.scalar.activation(out=h1[:, col0:col0 + 512],
                                         in_=h1_ps[...], func=AFT.Copy,
                                         accum_out=s1p[:, sidx:sidx + 1])
                    sq = stag.tile([128, 512], F32, tag="sq")
                    nc.scalar.activation(out=sq[...], in_=h1_ps[...],
                                         func=AFT.Square,
                                         accum_out=q1p[:, sidx:sidx + 1])

        # --- BN (exact global stats; partials already accumulated) ---------
        def bn_stats_apply(src, parts, sp, qp, g_col, b_col, relu_out, cc_name):
            s1 = singles.tile([parts, 1], F32, tag=cc_name + "_s1")
            nc.vector.tensor_reduce(out=s1[...], in_=sp,
                                    axis=mybir.AxisListType.X, op=AOP.add)
            q1 = singles.tile([parts, 1], F32, tag=cc_name + "_q1")
            nc.vector.tensor_reduce(out=q1[...], in_=qp,
                                    axis=mybir.AxisListType.X, op=AOP.add)
            if with_collective:
                cc_in = dram.tile([parts, 2], F32, tag=cc_name + "_in")
                cc_out = nc.dram_tensor(cc_name + "_out", (parts, 2), F32,
                                        kind="Internal", addr_space="Shared")
                nc.sync.dma_start(out=cc_in[:, 0:1], in_=s1[...])
                nc.sync.dma_start(out=cc_in[:, 1:2], in_=q1[...])
                nc.gpsimd.collective_compute(
                    "AllReduce", AOP.add,
                    replica_groups=[list(range(n_cores))],
                    ins=[cc_in[:, :].opt()], outs=[cc_out[:, :].opt()])
                sq = singles.tile([parts, 2], F32, tag=cc_name + "_sq")
                nc.sync.dma_start(out=sq[...], in_=cc_out[:, :])
                s1g, q1g = sq[:, 0:1], sq[:, 1:2]
            else:
                s1g, q1g = s1[:, :], q1[:, :]
            mean = singles.tile([parts, 1], F32, tag=cc_name + "_mean")
            nc.vector.tensor_scalar_mul(mean[...], s1g, 1.0 / btot)
            msq = singles.tile([parts, 1], F32, tag=cc_name + "_msq")
            nc.vector.tensor_scalar_mul(msq[...], q1g, 1.0 / btot)
            m2 = singles.tile([parts, 1], F32, tag=cc_name + "_m2")
            nc.vector.tensor_tensor(out=m2[...], in0=mean[...], in1=mean[...],
                                    op=AOP.mult)
            var = singles.tile([parts, 1], F32, tag=cc_name + "_var")
            nc.vector.tensor_tensor(out=var[...], in0=msq[...], in1=m2[...],
                                    op=AOP.subtract)
            eps = singles.tile([parts, 1], F32, tag=cc_name + "_eps")
            nc.vector.memset(eps[...], 1e-5)
            std = singles.tile([parts, 1], F32, tag=cc_name + "_std")
            nc.scalar.activation(out=std[...], in_=var[...], func=AFT.Sqrt,
                                 bias=eps[...])
            rstd = singles.tile([parts, 1], F32, tag=cc_name + "_rstd")
            nc.vector.reciprocal(out=rstd[...], in_=std[...])
            scale = singles.tile([parts, 1], F32, tag=cc_name + "_scale")
            nc.vector.tensor_tensor(out=scale[...], in0=g_col[...],
                                    in1=rstd[...], op=AOP.mult)
            mscale = singles.tile([parts, 1], F32, tag=cc_name + "_ms")
            nc.vector.tensor_tensor(out=mscale[...], in0=mean[...],
                                    in1=scale[...], op=AOP.mult)
            nbias = singles.tile([parts, 1], F32, tag=cc_name + "_nb")
            nc.vector.tensor_tensor(out=nbias[...], in0=b_col[...],
                                    in1=mscale[...], op=AOP.subtract)
            nc.scalar.activation(out=relu_out, in_=src, func=AFT.Relu,
                                 bias=nbias[...], scale=scale[...])

        bn_stats_apply(h1[:, :], 128, s1p[:, :], q1p[:, :], c_g1, c_b1,
                       a1[:, :], "cc1")

        with tc.tile_pool(name="ps_h2", bufs=2, space="PSUM") as ps_h2, \
             tc.tile_pool(name="h2st", bufs=2) as h2st:
            for sc2 in range(b // 512):
                h2_ps = ps_h2.tile([64, 512], F32, tag="h2ps")
                nc.tensor.matmul(h2_ps[...], c_w2T[:, :],
                                 a1[:, sc2 * 512:(sc2 + 1) * 512],
                                 start=True, stop=True)
                nc.scalar.activation(out=h2[:, sc2 * 512:(sc2 + 1) * 512],
                                     in_=h2_ps[...], func=AFT.Copy,
                                     accum_out=s2p[:, sc2:sc2 + 1])
                sq2 = h2st.tile([64, 512], F32, tag="sq2")
                nc.scalar.activation(out=sq2[...], in_=h2_ps[...],
                                     func=AFT.Square,
                                     accum_out=q2p[:, sc2:sc2 + 1])

        bn_stats_apply(h2[:, :], 64, s2p[:, :], q2p[:, :], c_g2, c_b2,
                       a2aug[0:64, :], "cc2")

        # --- P3: fc3 + counter + writeback ---------------------------------
        with tc.tile_pool(name="ps_o", bufs=4, space="PSUM") as ps_o, \
             tc.tile_pool(name="ctp", bufs=3) as ctpool, \
             tc.tile_pool(name="dg", bufs=2) as dgpool, \
             tc.tile_pool(name="ost", bufs=2) as ost:
            out_r = out_t[:, :].rearrange("(g p) n -> p g n", p=128)
            for ch in range(NCH3):
                ct = ctpool.tile([128, G3, 2, NB], BF16, tag="ct")
                c0 = ch * G3 * 2 * NB
                nc.sync.dma_start(out=ct[...], in_=ct_in[:, c0:c0 + G3 * 2 * NB])
                ostg = ost.tile([128, G3, NB], BF16, tag="ostg")
                for g in range(G3):
                    gg = ch * G3 + g
                    diag = dgpool.tile([128, 128], BF16, tag="diag")
                    nc.vector.tensor_scalar(
                        out=diag[...], in0=ident[:, :],
                        scalar1=c_rv[:, gg:gg + 1], scalar2=None, op0=AOP.mult)
                    o_ps = ps_o.tile([128, NB], F32, tag="ops")
                    nc.tensor.matmul(o_ps[...],
                                     a2aug[:, gg * 128:(gg + 1) * 128],
                                     c_w3aug[:, :], start=True, stop=False)
                    nc.tensor.matmul(o_ps[...], diag[...], ct[:, g, 0, :],
                                     start=False, stop=False)
                    nc.tensor.matmul(o_ps[...], diag[...], ct[:, g, 1, :],
                                     start=False, stop=True)
                    nc.scalar.activation(out=ostg[:, g], in_=o_ps[...],
                                         func=AFT.Copy)
                nc.sync.dma_start(out=out_r[:, ch * G3:(ch + 1) * G3, :],
                                  in_=ostg[...])

    nc.compile()
    return nc


# ---------------------------------------------------------------------------
# host wrapper
# ---------------------------------------------------------------------------

_NC_CACHE = {}


def make_core_inputs(inputs, pc, b, n_cores):
    import ml_dtypes
    bf16 = ml_dtypes.bfloat16
    friends = np.asarray(inputs['friends'], np.int64)
    enemies = np.asarray(inputs['enemies'], np.int64)
    map_idx = np.asarray(inputs['map_idx'], np.int64)
    n = friends.shape[0]

    # bf16 tables: host gathers are then raw row copies
    TEA = pc['T_EA'].reshape(3 * NB, 3 * NB, NH).astype(bf16)
    TFA = pc['T_FA'].astype(bf16)
    TCA = pc['TCA'].astype(bf16)
    v_ea = pc['v_ea'].astype(bf16)
    v_fa = pc['v_fa'].astype(bf16)
    m_tab = pc['m_tab'].astype(bf16)
    cn = pc['cn'].astype(np.float32)
    P2 = (cn[:, None, :] + cn[None, :, :]).astype(bf16).reshape(NB * NB, NB)
    cn16 = cn.astype(bf16)

    # per-sample record [S_ea | S_fa | Tt | v_e | v_f | mrow]
    rec = np.empty((n, REC), bf16)
    ei = np.arange(3)[None] * NB + enemies                  # (n, 3) token idx
    # scores stored (j_key, i_query, h): TEA[A,B][b,x,y] = s(query=y, key=x)
    rec[:, OFF_SEA:OFF_SFA] = TEA[ei[:, None, :], ei[:, :, None]].reshape(n, 36)
    rec[:, OFF_SFA:OFF_T] = TFA[friends[:, None, :], friends[:, :, None]] \
        .reshape(n, 36)
    la = np.concatenate([friends, np.full((n, 1), NB)], 1)
    mb = np.concatenate([ei, np.full((n, 1), 3 * NB)], 1)
    rec[:, OFF_T:OFF_VE] = TCA[la[:, :, None], mb[:, None, :]].reshape(n, 64)
    rec[:, OFF_VE:OFF_VF] = v_ea[np.arange(3)[None], enemies].reshape(n, 96)
    rec[:, OFF_VF:OFF_MR] = v_fa[friends].reshape(n, 96)
    rec[:, OFF_MR:] = m_tab[map_idx[:, 0]]

    ctg = np.empty((n, 2, NB), bf16)
    ctg[:, 0] = P2[enemies[:, 0] * NB + enemies[:, 1]]
    ctg[:, 1] = cn16[enemies[:, 2]]
    valid = (enemies != 0).sum(1)
    rv = np.where(valid > 0, 1.0 / np.maximum(valid, 1), 0.0).astype(np.float32)

    shared = dict(
        dbT=pc['dbT'].astype(bf16), w2T=pc['w2T'].astype(bf16),
        w3aug=pc['w3aug'].astype(bf16),
        bn_g1=pc['bn1_g'].reshape(-1, 1).astype(np.float32),
        bn_b1=pc['bn1_b'].reshape(-1, 1).astype(np.float32),
        bn_g2=pc['bn2_g'].reshape(-1, 1).astype(np.float32),
        bn_b2=pc['bn2_b'].reshape(-1, 1).astype(np.float32),
    )

    in_maps = []
    for c in range(n_cores):
        lo, hi = c * b, (c + 1) * b
        m = dict(shared)
        m['xrec'] = np.ascontiguousarray(
            rec[lo:hi].reshape(b // 128, 128, REC).transpose(1, 0, 2)
        ).reshape(128, -1)
        m['ct'] = np.ascontiguousarray(
            ctg[lo:hi].reshape(b // 128, 128, 2 * NB).transpose(1, 0, 2)
        ).reshape(128, -1)
        m['rv'] = np.ascontiguousarray(
            rv[lo:hi].reshape(b // 128, 128).T)
        in_maps.append(m)
    return in_maps


def kernel(**inputs):
    from concourse.bass_utils import run_bass_kernel_spmd
    b = B_FULL // NCORES
    pc = host_precompute(inputs)
    key = (b, NCORES)
    if key not in _NC_CACHE:
        _NC_CACHE[key] = build_nc(b, NCORES, with_collective=True)
    nc = _NC_CACHE[key]
    in_maps = make_core_inputs(inputs, pc, b, NCORES)
    res = run_bass_kernel_spmd(nc, in_maps, core_ids=list(range(NCORES)))
    out = np.concatenate([np.asarray(r['out'], np.float32)
                          for r in res.results], 0)
    return out


# revision 10
# speedup vs baseline: 2.5569x; 1.1436x over previous
"""Trainium2 Bass kernel for nn_EnhancedBrawlerPredictionModel (B=65536).

Data-parallel over 8 NeuronCores (8192 samples/core). All parameter algebra is
folded on the host into input-independent lookup tables; per-sample HOST work
is index-gathers only (no per-sample arithmetic). The device computes the
three softmaxes, the attention-weighted sums, the cross-attention bilinear
score contraction, fc1/fc2/fc3, exact-global-batch BatchNorm (two tiny
AllReduces), and the counter-matrix influence.

Table folding:
  - self-attention scores are bilinear in (token, embedding-id) pairs ->
    gathered per-sample from T_EA[(i,e_i),(j,e_j),h], T_FA[f_i,f_j,h].
  - cross-attention q/k are affine in the fa/ea attention *weights* ->
    s_ca = af~^T Tt ae~ with Tt a per-sample gathered 4x4x4 table
    (bias row/col augmented).
  - h1 = sum_i D_i z_i + sum_i B_i ao_e_i + m_tab[map] with
    z_i = sum_j a_ca[j,i] ao_e_j (Mv and fc1 blocks folded into D_i).
  - counter influence: unscaled pair-sum table P2[e1,e2] + single row,
    scaled by 1/valid on device via a diagonal matmul.
"""

import numpy as np

import concourse.bass as bass
import concourse.bacc as bacc
import concourse.tile as tile
import concourse.mybir as mybir
from concourse.masks import make_identity

F32 = mybir.dt.float32
BF16 = mybir.dt.bfloat16
I32 = mybir.dt.int32

B_FULL = 65536
NCORES = 8
E, NH, DH, S = 32, 4, 8, 3
NB, NM, H = 512, 128, 128
AOP = mybir.AluOpType
AFT = mybir.ActivationFunctionType

# per-sample record layout (bf16 elements)
OFF_SEA, OFF_SFA, OFF_T, OFF_VE, OFF_MR, REC = 0, 36, 72, 136, 232, 360

# ---------------------------------------------------------------------------
# host-side precompute (input-independent tables)
# ---------------------------------------------------------------------------


def host_precompute(inp):
    f32 = np.float32
    emb = np.asarray(inp['brawler_emb'], f32)
    pos_w = np.asarray(inp['pos_w'], f32)
    pos_b = np.asarray(inp['pos_b'], f32)
    pos_emb = np.arange(S, dtype=f32)[:, None] * pos_w[None, :, 0] + pos_b

    def split_in(w, b):
        w = np.asarray(w, f32)
        b = np.asarray(b, f32)
        return (w[:E], w[E:2 * E], w[2 * E:], b[:E], b[E:2 * E], b[2 * E:])

    Wq_ea, Wk_ea, Wv_ea, bq_ea, bk_ea, bv_ea = split_in(inp['ea_in_w'], inp['ea_in_b'])
    Wq_fa, Wk_fa, Wv_fa, bq_fa, bk_fa, bv_fa = split_in(inp['fa_in_w'], inp['fa_in_b'])
    Wq_ca, Wk_ca, Wv_ca, bq_ca, bk_ca, bv_ca = split_in(inp['ca_in_w'], inp['ca_in_b'])
    Wout_ea = np.asarray(inp['ea_out_w'], f32)
    bout_ea = np.asarray(inp['ea_out_b'], f32)
    Wout_fa = np.asarray(inp['fa_out_w'], f32)
    bout_fa = np.asarray(inp['fa_out_b'], f32)
    Wout_ca = np.asarray(inp['ca_out_w'], f32)

    t_ea = emb[None] + pos_emb[:, None]                  # (3, 512, 32)

    def hdot(A, B):
        return np.einsum('...hd,...hd->...h',
                         A.reshape(*A.shape[:-1], NH, DH),
                         B.reshape(*B.shape[:-1], NH, DH))

    q_ea = t_ea @ Wq_ea.T + bq_ea
    k_ea = t_ea @ Wk_ea.T + bk_ea
    T_EA = hdot(q_ea[:, :, None, None], k_ea[None, None])    # (3,512,3,512,4)
    q_fa = emb @ Wq_fa.T + bq_fa
    k_fa = emb @ Wk_fa.T + bk_fa
    T_FA = hdot(q_fa[:, None], k_fa[None])                   # (512, 512, 4)

    v_ea = t_ea @ Wv_ea.T                                    # (3, 512, 32)
    v_fa = emb @ Wv_fa.T                                     # (512, 32)

    Mq = Wq_ca @ Wout_fa
    bq_f = Mq @ bv_fa + Wq_ca @ bout_fa + bq_ca
    Mk = Wk_ca @ Wout_ea
    bk_f = Mk @ bv_ea + Wk_ca @ bout_ea + bk_ca
    Mv = Wv_ca @ Wout_ea

    TqA = np.concatenate([v_fa @ Mq.T, bq_f[None]], 0)       # (513, 32)
    TkA = np.concatenate([(v_ea @ Mk.T).reshape(3 * NB, E), bk_f[None]], 0)
    TCA = hdot(TqA[:, None], TkA[None])                      # (513, 1537, 4)

    fc1_w = np.asarray(inp['fc1_w'], f32)
    A_ca = fc1_w[:, 0:96].reshape(H, 3, E)
    A_ea = fc1_w[:, 96:192].reshape(H, 3, E)
    A_m = fc1_w[:, 192:224]
    m_tab = np.asarray(inp['map_emb'], f32) @ A_m.T          # (128, 128)

    # db[i,:,0] = D_i.T (Mv folded), db[i,:,1] = B_i.T
    db = np.zeros((3, 32, 2, 128), f32)
    for i in range(3):
        db[i, :, 0] = ((A_ca[:, i] @ Wout_ca) @ Mv).T
        db[i, :, 1] = (A_ea[:, i] @ Wout_ea).T

    counter = np.asarray(inp['counter_matrix'], f32)
    nz = (np.arange(NB) != 0).astype(f32)
    cn = nz[:, None] * counter                               # (512, 512)

    W3aug = np.concatenate([np.asarray(inp['fc3_w'], f32).T,
                            np.asarray(inp['fc3_b'], f32)[None, :]], 0)

    return dict(
        T_EA=T_EA, T_FA=T_FA, TCA=TCA, v_ea=v_ea, v_fa=v_fa, m_tab=m_tab,
        cn=cn, dbT=db.reshape(96, 2, 128), w2T=np.asarray(inp['fc2_w'], f32).T,
        w3aug=W3aug,
        bn1_g=np.asarray(inp['bn1_g'], f32), bn1_b=np.asarray(inp['bn1_b'], f32),
        bn2_g=np.asarray(inp['bn2_g'], f32), bn2_b=np.asarray(inp['bn2_b'], f32),
    )


# ---------------------------------------------------------------------------
# device kernel
# ---------------------------------------------------------------------------


def build_nc(b, n_cores, with_collective=True, stage=5):
    assert b % 1024 == 0
    nc = bacc.Bacc("TRN2", target_bir_lowering=False, debug=False,
                   num_devices=n_cores)

    G1 = 16                    # P1 chunk = 2048 samples
    CH1 = G1 * 128
    NCH1 = b // CH1
    G3 = 8                     # P3 chunk = 1024 samples
    CH3 = G3 * 128
    NCH3 = b // CH3
    NGG = b // 128             # total 128-sample groups
    btot = float(b * (n_cores if with_collective else 1))
    rsq = float(1.0 / np.sqrt(DH))

    dt_i = nc.dram_tensor
    xrec = dt_i("xrec", (128, NGG * REC), BF16, kind="ExternalInput")
    ct_in = dt_i("ct", (128, NGG * 2 * NB), BF16, kind="ExternalInput")
    rv_in = dt_i("rv", (128, NGG), F32, kind="ExternalInput")
    dbT = dt_i("dbT", (96, 2, 128), BF16, kind="ExternalInput")
    w2T = dt_i("w2T", (128, 64), BF16, kind="ExternalInput")
    w3aug = dt_i("w3aug", (65, NB), BF16, kind="ExternalInput")
    bn_g1 = dt_i("bn_g1", (H, 1), F32, kind="ExternalInput")
    bn_b1 = dt_i("bn_b1", (H, 1), F32, kind="ExternalInput")
    bn_g2 = dt_i("bn_g2", (64, 1), F32, kind="ExternalInput")
    bn_b2 = dt_i("bn_b2", (64, 1), F32, kind="ExternalInput")
    out_t = dt_i("out", (b, NB), BF16, kind="ExternalOutput")

    import contextlib
    with tile.TileContext(nc) as tc, contextlib.ExitStack() as ctx:
        singles = ctx.enter_context(tc.tile_pool(name="singles", bufs=1))
        dram = ctx.enter_context(tc.tile_pool(name="dram", bufs=1, space="DRAM"))

        ident = singles.tile([128, 128], BF16)
        make_identity(nc, ident[:, :])

        def load(name, shape, dtype, src):
            t = singles.tile(shape, dtype, tag="c_" + name)
            nc.sync.dma_start(out=t[...], in_=src)
            return t

        c_dbT = load("dbT", [96, 2, 128], BF16, dbT[:, :, :])
        c_w2T = load("w2T", [128, 64], BF16, w2T[:, :])
        c_w3aug = load("w3aug", [65, NB], BF16, w3aug[:, :])
        c_g1 = load("g1", [H, 1], F32, bn_g1[:, :])
        c_b1 = load("b1", [H, 1], F32, bn_b1[:, :])
        c_g2 = load("g2", [64, 1], F32, bn_g2[:, :])
        c_b2 = load("b2", [64, 1], F32, bn_b2[:, :])
        c_rv = load("rv", [128, NGG], F32, rv_in[:, :])

        h1 = singles.tile([128, b], BF16)
        h2 = singles.tile([64, b], BF16)
        a2aug = singles.tile([65, b], BF16)
        nc.vector.memset(a2aug[64:65, :], 1.0)
        # per-512-chunk BN stat partials (sum / sum-of-squares)
        s1p = singles.tile([128, b // 512], F32)
        q1p = singles.tile([128, b // 512], F32)
        s2p = singles.tile([64, b // 512], F32)
        q2p = singles.tile([64, b // 512], F32)

        # --- P1: softmax / AV / ca-bilinear chain + h1 ---------------------
        with tc.tile_pool(name="attn", bufs=1) as atp, \
             tc.tile_pool(name="expool", bufs=2) as expool, \
             tc.tile_pool(name="gath", bufs=3) as gath, \
             tc.tile_pool(name="ao", bufs=2) as aopool, \
             tc.tile_pool(name="stag", bufs=2) as stag, \
             tc.tile_pool(name="ps_t", bufs=1, space="PSUM") as ps_t, \
             tc.tile_pool(name="ps_h1", bufs=2, space="PSUM") as ps_h1:
            for ch in range(NCH1):
                xr = gath.tile([128, G1, REC], BF16, tag="xr")
                nc.sync.dma_start(out=xr[...],
                                  in_=xrec[:, ch * G1 * REC:(ch + 1) * G1 * REC])

                # ea+fa softmax from gathered scores (layout w, j(key), i, h)
                e_b = expool.tile([128, G1, 2, 3, 12], F32, tag="e_b")
                nc.scalar.activation(
                    out=e_b.rearrange("p g w j x -> p g (w j x)"),
                    in_=xr[:, :, 0:72], func=AFT.Exp, scale=rsq)
                den = atp.tile([128, G1, 2, 12], F32, tag="den")
                nc.vector.tensor_tensor(out=den, in0=e_b[:, :, :, 0],
                                        in1=e_b[:, :, :, 1], op=AOP.add)
                den2 = atp.tile([128, G1, 2, 12], F32, tag="den2")
                nc.vector.tensor_tensor(out=den2, in0=den, in1=e_b[:, :, :, 2],
                                        op=AOP.add)
                r_b = atp.tile([128, G1, 2, 12], F32, tag="r_b")
                rs = atp.tile([128, G1, 2, 12], F32, tag="rs")
                nc.vector.reciprocal_approx_accurate(
                    out=r_b.rearrange("p g w x -> p (g w x)"),
                    in_=den2.rearrange("p g w x -> p (g w x)"),
                    scratch=rs.rearrange("p g w x -> p (g w x)"))
                a_b = atp.tile([128, G1, 2, 3, 12], BF16, tag="a_b")
                for w in range(2):
                    nc.vector.tensor_tensor(
                        out=a_b[:, :, w], in0=e_b[:, :, w],
                        in1=r_b[:, :, w].unsqueeze(2)
                        .to_broadcast([128, G1, 3, 12]),
                        op=AOP.mult)

                # AV for ea and fa: ao[i] = sum_j a[j,i] (x) v[j]
                ao_e = aopool.tile([128, G1, 3, E], BF16, tag="ao_e")

                def attn_av(w, a_t, v_of_j, ao):
                    # v_of_j(j) -> [128, G1, NH, DH]-broadcastable AP
                    avs = []
                    for j in range(3):
                        av = atp.tile([128, G1, 3, NH, DH], BF16,
                                      tag=f"av{w}{j}")
                        v_in = v_of_j(j)
                        for i in range(3):
                            a_in = a_t[:, :, j, 4 * i:4 * (i + 1)] \
                                .unsqueeze(3).to_broadcast([128, G1, NH, DH])
                            nc.vector.tensor_tensor(out=av[:, :, i],
                                                    in0=a_in, in1=v_in,
                                                    op=AOP.mult)
                        avs.append(av)
                    s01 = atp.tile([128, G1, 3, NH, DH], BF16, tag=f"s01{w}")
                    nc.vector.tensor_tensor(out=s01[...], in0=avs[0][...],
                                            in1=avs[1][...], op=AOP.add)
                    nc.vector.tensor_tensor(
                        out=ao.rearrange("p g i (h d) -> p g i h d", d=DH),
                        in0=s01[...], in1=avs[2][...], op=AOP.add)

                attn_av(0, a_b[:, :, 0],
                        lambda j: xr[:, :, OFF_VE + E * j:OFF_VE + E * (j + 1)]
                        .rearrange("p g (h d) -> p g h d", d=DH), ao_e)

                # ca bilinear scores: s[j,i,h] = af~_i^T Tt ae~_j
                # U[i,m,h] = sum_l af[l,i,h] Tt[l,m,h]  (+ bias row l'=3)
                us = []
                for l in range(3):
                    u = atp.tile([128, G1, 3, 4, NH], BF16, tag=f"u{l}")
                    tl = xr[:, :, OFF_T + 16 * l:OFF_T + 16 * (l + 1)] \
                        .rearrange("p g (m h) -> p g m h", h=NH)
                    for i in range(3):
                        af = a_b[:, :, 1, l, 4 * i:4 * (i + 1)] \
                            .unsqueeze(2).to_broadcast([128, G1, 4, NH])
                        nc.vector.tensor_tensor(out=u[:, :, i], in0=af,
                                                in1=tl, op=AOP.mult)
                    us.append(u)
                t3b = xr[:, :, OFF_T + 48:OFF_T + 64].rearrange(
                    "p g (m h) -> p g m h", h=NH).unsqueeze(2) \
                    .to_broadcast([128, G1, 3, 4, NH])
                us1 = atp.tile([128, G1, 3, 4, NH], BF16, tag="us1")
                us2 = atp.tile([128, G1, 3, 4, NH], BF16, tag="us2")
                U = atp.tile([128, G1, 3, 4, NH], BF16, tag="U")
                nc.vector.tensor_tensor(out=us1[...], in0=us[0][...],
                                        in1=us[1][...], op=AOP.add)
                nc.vector.tensor_tensor(out=us2[...], in0=us[2][...], in1=t3b,
                                        op=AOP.add)
                nc.vector.tensor_tensor(out=U[...], in0=us1[...], in1=us2[...],
                                        op=AOP.add)

                # s_ca[j,i,h] = sum_m U[i,m,h] ae[m,j,h]  (+ bias col m'=3)
                sc = atp.tile([128, G1, 3, 3, NH], BF16, tag="sc")
                for i in range(3):
                    vs = []
                    for m in range(3):
                        v = atp.tile([128, G1, 3, NH], BF16, tag=f"vv{i}{m}")
                        ui = U[:, :, i, m, :].unsqueeze(2) \
                            .to_broadcast([128, G1, 3, NH])
                        ae = a_b[:, :, 0, m].rearrange(
                            "p g (j2 h) -> p g j2 h", h=NH)
                        nc.vector.tensor_tensor(out=v[...], in0=ui, in1=ae,
                                                op=AOP.mult)
                        vs.append(v)
                    u3 = U[:, :, i, 3, :].unsqueeze(2) \
                        .to_broadcast([128, G1, 3, NH])
                    vs1 = atp.tile([128, G1, 3, NH], BF16, tag=f"vs1{i}")
                    vs2 = atp.tile([128, G1, 3, NH], BF16, tag=f"vs2{i}")
                    nc.vector.tensor_tensor(out=vs1[...], in0=vs[0][...],
                                            in1=vs[1][...], op=AOP.add)
                    nc.vector.tensor_tensor(out=vs2[...], in0=vs[2][...],
                                            in1=u3, op=AOP.add)
                    nc.vector.tensor_tensor(out=sc[:, :, :, i, :],
                                            in0=vs1[...], in1=vs2[...],
                                            op=AOP.add)

                # ca softmax
                e_c = expool.tile([128, G1, 3, 12], F32, tag="e_c")
                nc.scalar.activation(
                    out=e_c.rearrange("p g j x -> p g (j x)"),
                    in_=sc.rearrange("p g j i h -> p g (j i h)"),
                    func=AFT.Exp, scale=rsq)
                dc = atp.tile([128, G1, 12], F32, tag="dc")
                nc.vector.tensor_tensor(out=dc, in0=e_c[:, :, 0],
                                        in1=e_c[:, :, 1], op=AOP.add)
                dc2 = atp.tile([128, G1, 12], F32, tag="dc2")
                nc.vector.tensor_tensor(out=dc2, in0=dc, in1=e_c[:, :, 2],
                                        op=AOP.add)
                r_c = atp.tile([128, G1, 12], F32, tag="r_c")
                rcs = atp.tile([128, G1, 12], F32, tag="rcs")
                nc.vector.reciprocal_approx_accurate(
                    out=r_c.rearrange("p g x -> p (g x)"),
                    in_=dc2.rearrange("p g x -> p (g x)"),
                    scratch=rcs.rearrange("p g x -> p (g x)"))
                a_c = atp.tile([128, G1, 3, 12], BF16, tag="a_c")
                nc.vector.tensor_tensor(
                    out=a_c, in0=e_c,
                    in1=r_c.unsqueeze(2).to_broadcast([128, G1, 3, 12]),
                    op=AOP.mult)

                # z AV over ao_e
                z = aopool.tile([128, G1, 3, E], BF16, tag="z")
                attn_av(2, a_c[:, :, :, :],
                        lambda j: ao_e[:, :, j].rearrange(
                            "p g (h d) -> p g h d", d=DH), z)

                # per 512-sample sub-chunk: transposes + h1 accumulation
                for sub in range(G1 // 4):
                    g0 = sub * 4
                    sidx = ch * (G1 // 4) + sub
                    col0 = sidx * 512
                    aoeT_ps = ps_t.tile([96, 512], BF16, tag="aoeT")
                    zT_ps = ps_t.tile([96, 512], BF16, tag="zT")
                    for t in range(4):
                        nc.tensor.transpose(
                            aoeT_ps[:, t * 128:(t + 1) * 128],
                            ao_e[:, g0 + t].rearrange("p i d -> p (i d)"),
                            ident[:, :])
                        nc.tensor.transpose(
                            zT_ps[:, t * 128:(t + 1) * 128],
                            z[:, g0 + t].rearrange("p i d -> p (i d)"),
                            ident[:, :])
                    aoeT = stag.tile([96, 512], BF16, tag="aoeT_s")
                    zT = stag.tile([96, 512], BF16, tag="zT_s")
                    nc.scalar.activation(out=aoeT[...], in_=aoeT_ps[...],
                                         func=AFT.Copy)
                    nc.scalar.activation(out=zT[...], in_=zT_ps[...],
                                         func=AFT.Copy)

                    h1_ps = ps_h1.tile([128, 512], F32, tag="h1ps")
                    nc.tensor.matmul(h1_ps[...], c_dbT[:, 0, :], zT[...],
                                     start=True, stop=False)
                    nc.tensor.matmul(h1_ps[...], c_dbT[:, 1, :], aoeT[...],
                                     start=False, stop=False)
                    for t in range(4):
                        nc.tensor.matmul(
                            h1_ps[:, t * 128:(t + 1) * 128],
                            xr[:, g0 + t, OFF_MR:OFF_MR + 128],
                            ident[:, :], start=False, stop=(t == 3))
                    nc.scalar.activation(out=h1[:, col0:col0 + 512],
                                         in_=h1_ps[...], func=AFT.Copy,
                                         accum_out=s1p[:, sidx:sidx + 1])
                    sq = stag.tile([128, 512], F32, tag="sq")
                    nc.scalar.activation(out=sq[...], in_=h1_ps[...],
                                         func=AFT.Square,
                                         accum_out=q1p[:, sidx:sidx + 1])

        # --- BN (exact global stats; partials already accumulated) ---------
        def bn_stats(parts, sp, qp, g_col, b_col, cc_name):
            s1 = singles.tile([parts, 1], F32, tag=cc_name + "_s1")
            nc.vector.tensor_reduce(out=s1[...], in_=sp,
                                    axis=mybir.AxisListType.X, op=AOP.add)
            q1 = singles.tile([parts, 1], F32, tag=cc_name + "_q1")
            nc.vector.tensor_reduce(out=q1[...], in_=qp,
                                    axis=mybir.AxisListType.X, op=AOP.add)
            if with_collective:
                cc_in = dram.tile([parts, 2], F32, tag=cc_name + "_in")
                cc_out = nc.dram_tensor(cc_name + "_out", (parts, 2), F32,
                                        kind="Internal", addr_space="Shared")
                nc.sync.dma_start(out=cc_in[:, 0:1], in_=s1[...])
                nc.sync.dma_start(out=cc_in[:, 1:2], in_=q1[...])
                nc.gpsimd.collective_compute(
                    "AllReduce", AOP.add,
                    replica_groups=[list(range(n_cores))],
                    ins=[cc_in[:, :].opt()], outs=[cc_out[:, :].opt()])
                sq = singles.tile([parts, 2], F32, tag=cc_name + "_sq")
                nc.sync.dma_start(out=sq[...], in_=cc_out[:, :])
                s1g, q1g = sq[:, 0:1], sq[:, 1:2]
            else:
                s1g, q1g = s1[:, :], q1[:, :]
            mean = singles.tile([parts, 1], F32, tag=cc_name + "_mean")
            nc.vector.tensor_scalar_mul(mean[...], s1g, 1.0 / btot)
            msq = singles.tile([parts, 1], F32, tag=cc_name + "_msq")
            nc.vector.tensor_scalar_mul(msq[...], q1g, 1.0 / btot)
            m2 = singles.tile([parts, 1], F32, tag=cc_name + "_m2")
            nc.vector.tensor_tensor(out=m2[...], in0=mean[...], in1=mean[...],
                                    op=AOP.mult)
            var = singles.tile([parts, 1], F32, tag=cc_name + "_var")
            nc.vector.tensor_tensor(out=var[...], in0=msq[...], in1=m2[...],
                                    op=AOP.subtract)
            eps = singles.tile([parts, 1], F32, tag=cc_name + "_eps")
            nc.vector.memset(eps[...], 1e-5)
            std = singles.tile([parts, 1], F32, tag=cc_name + "_std")
            nc.scalar.activation(out=std[...], in_=var[...], func=AFT.Sqrt,
                                 bias=eps[...])
            rstd = singles.tile([parts, 1], F32, tag=cc_name + "_rstd")
            nc.vector.reciprocal(out=rstd[...], in_=std[...])
            scale = singles.tile([parts, 1], F32, tag=cc_name + "_scale")
            nc.vector.tensor_tensor(out=scale[...], in0=g_col[...],
                                    in1=rstd[...], op=AOP.mult)
            mscale = singles.tile([parts, 1], F32, tag=cc_name + "_ms")
            nc.vector.tensor_tensor(out=mscale[...], in0=mean[...],
                                    in1=scale[...], op=AOP.mult)
            nbias = singles.tile([parts, 1], F32, tag=cc_name + "_nb")
            nc.vector.tensor_tensor(out=nbias[...], in0=b_col[...],
                                    in1=mscale[...], op=AOP.subtract)
            return scale, nbias

        sc1, nb1 = bn_stats(128, s1p[:, :], q1p[:, :], c_g1, c_b1, "cc1")

        with tc.tile_pool(name="ps_h2", bufs=2, space="PSUM") as ps_h2, \
             tc.tile_pool(name="h2st", bufs=2) as h2st:
            for sc2 in range(b // 512):
                a1c = h2st.tile([128, 512], BF16, tag="a1c")
                nc.scalar.activation(out=a1c[...],
                                     in_=h1[:, sc2 * 512:(sc2 + 1) * 512],
                                     func=AFT.Relu, bias=nb1[...],
                                     scale=sc1[...])
                h2_ps = ps_h2.tile([64, 512], F32, tag="h2ps")
                nc.tensor.matmul(h2_ps[...], c_w2T[:, :], a1c[...],
                                 start=True, stop=True)
                nc.scalar.activation(out=h2[:, sc2 * 512:(sc2 + 1) * 512],
                                     in_=h2_ps[...], func=AFT.Copy,
                                     accum_out=s2p[:, sc2:sc2 + 1])
                sq2 = h2st.tile([64, 512], F32, tag="sq2")
                nc.scalar.activation(out=sq2[...], in_=h2_ps[...],
                                     func=AFT.Square,
                                     accum_out=q2p[:, sc2:sc2 + 1])

        sc2_, nb2_ = bn_stats(64, s2p[:, :], q2p[:, :], c_g2, c_b2, "cc2")
        nc.scalar.activation(out=a2aug[0:64, :], in_=h2[:, :], func=AFT.Relu,
                             bias=nb2_[...], scale=sc2_[...])

        # --- P3: fc3 + counter + writeback ---------------------------------
        with tc.tile_pool(name="ps_o", bufs=4, space="PSUM") as ps_o, \
             tc.tile_pool(name="ctp", bufs=3) as ctpool, \
             tc.tile_pool(name="dg", bufs=2) as dgpool, \
             tc.tile_pool(name="ost", bufs=2) as ost:
            out_r = out_t[:, :].rearrange("(g p) n -> p g n", p=128)
            for ch in range(NCH3):
                ct = ctpool.tile([128, G3, 2, NB], BF16, tag="ct")
                c0 = ch * G3 * 2 * NB
                nc.sync.dma_start(out=ct[...], in_=ct_in[:, c0:c0 + G3 * 2 * NB])
                ostg = ost.tile([128, G3, NB], BF16, tag="ostg")
                for g in range(G3):
                    gg = ch * G3 + g
                    diag = dgpool.tile([128, 128], BF16, tag="diag")
                    nc.vector.tensor_scalar(
                        out=diag[...], in0=ident[:, :],
                        scalar1=c_rv[:, gg:gg + 1], scalar2=None, op0=AOP.mult)
                    o_ps = ps_o.tile([128, NB], F32, tag="ops")
                    nc.tensor.matmul(o_ps[...],
                                     a2aug[:, gg * 128:(gg + 1) * 128],
                                     c_w3aug[:, :], start=True, stop=False)
                    nc.tensor.matmul(o_ps[...], diag[...], ct[:, g, 0, :],
                                     start=False, stop=False)
                    nc.tensor.matmul(o_ps[...], diag[...], ct[:, g, 1, :],
                                     start=False, stop=True)
                    nc.scalar.activation(out=ostg[:, g], in_=o_ps[...],
                                         func=AFT.Copy)
                nc.sync.dma_start(out=out_r[:, ch * G3:(ch + 1) * G3, :],
                                  in_=ostg[...])

    nc.compile()
    return nc


# ---------------------------------------------------------------------------
# host wrapper
# ---------------------------------------------------------------------------

_NC_CACHE = {}


def make_core_inputs(inputs, pc, b, n_cores):
    import ml_dtypes
    bf16 = ml_dtypes.bfloat16
    friends = np.asarray(inputs['friends'], np.int64)
    enemies = np.asarray(inputs['enemies'], np.int64)
    map_idx = np.asarray(inputs['map_idx'], np.int64)
    n = friends.shape[0]

    # bf16 tables: host gathers are then raw row copies
    TEA = pc['T_EA'].reshape(3 * NB, 3 * NB, NH).astype(bf16)
    TFA = pc['T_FA'].astype(bf16)
    TCA = pc['TCA'].astype(bf16)
    v_ea = pc['v_ea'].astype(bf16)
    m_tab = pc['m_tab'].astype(bf16)
    cn = pc['cn'].astype(np.float32)
    P2 = (cn[:, None, :] + cn[None, :, :]).astype(bf16).reshape(NB * NB, NB)
    cn16 = cn.astype(bf16)

    # per-sample record [S_ea | S_fa | Tt | v_e | v_f | mrow]
    rec = np.empty((n, REC), bf16)
    ei = np.arange(3)[None] * NB + enemies                  # (n, 3) token idx
    # scores stored (j_key, i_query, h): TEA[A,B][b,x,y] = s(query=y, key=x)
    rec[:, OFF_SEA:OFF_SFA] = TEA[ei[:, None, :], ei[:, :, None]].reshape(n, 36)
    rec[:, OFF_SFA:OFF_T] = TFA[friends[:, None, :], friends[:, :, None]] \
        .reshape(n, 36)
    la = np.concatenate([friends, np.full((n, 1), NB)], 1)
    mb = np.concatenate([ei, np.full((n, 1), 3 * NB)], 1)
    rec[:, OFF_T:OFF_VE] = TCA[la[:, :, None], mb[:, None, :]].reshape(n, 64)
    rec[:, OFF_VE:OFF_MR] = v_ea[np.arange(3)[None], enemies].reshape(n, 96)
    rec[:, OFF_MR:] = m_tab[map_idx[:, 0]]

    ctg = np.empty((n, 2, NB), bf16)
    ctg[:, 0] = P2[enemies[:, 0] * NB + enemies[:, 1]]
    ctg[:, 1] = cn16[enemies[:, 2]]
    valid = (enemies != 0).sum(1)
    rv = np.where(valid > 0, 1.0 / np.maximum(valid, 1), 0.0).astype(np.float32)

    shared = dict(
        dbT=pc['dbT'].astype(bf16), w2T=pc['w2T'].astype(bf16),
        w3aug=pc['w3aug'].astype(bf16),
        bn_g1=pc['bn1_g'].reshape(-1, 1).astype(np.float32),
        bn_b1=pc['bn1_b'].reshape(-1, 1).astype(np.float32),
        bn_g2=pc['bn2_g'].reshape(-1, 1).astype(np.float32),
        bn_b2=pc['bn2_b'].reshape(-1, 1).astype(np.float32),
    )

    in_maps = []
    for c in range(n_cores):
        lo, hi = c * b, (c + 1) * b
        m = dict(shared)
        m['xrec'] = np.ascontiguousarray(
            rec[lo:hi].reshape(b // 128, 128, REC).transpose(1, 0, 2)
        ).reshape(128, -1)
        m['ct'] = np.ascontiguousarray(
            ctg[lo:hi].reshape(b // 128, 128, 2 * NB).transpose(1, 0, 2)
        ).reshape(128, -1)
        m['rv'] = np.ascontiguousarray(
            rv[lo:hi].reshape(b // 128, 128).T)
        in_maps.append(m)
    return in_maps


def kernel(**inputs):
    from concourse.bass_utils import run_bass_kernel_spmd
    b = B_FULL // NCORES
    pc = host_precompute(inputs)
    key = (b, NCORES)
    if key not in _NC_CACHE:
        _NC_CACHE[key] = build_nc(b, NCORES, with_collective=True)
    nc = _NC_CACHE[key]
    in_maps = make_core_inputs(inputs, pc, b, NCORES)
    res = run_bass_kernel_spmd(nc, in_maps, core_ids=list(range(NCORES)))
    out = np.concatenate([np.asarray(r['out'], np.float32)
                          for r in res.results], 0)
    return out


# revision 12
# speedup vs baseline: 3.0116x; 1.1779x over previous
"""Trainium2 Bass kernel for nn_EnhancedBrawlerPredictionModel (B=65536).

Data-parallel over 8 NeuronCores (8192 samples/core). All parameter algebra is
folded on the host into input-independent lookup tables; per-sample HOST work
is index-gathers only (no per-sample arithmetic). The device computes the
three softmaxes, the attention-weighted sums, the cross-attention bilinear
score contraction, fc1/fc2/fc3, exact-global-batch BatchNorm (two tiny
AllReduces), and the counter-matrix influence.

Table folding:
  - self-attention scores are bilinear in (token, embedding-id) pairs ->
    gathered per-sample from T_EA[(i,e_i),(j,e_j),h], T_FA[f_i,f_j,h].
  - cross-attention q/k are affine in the fa/ea attention *weights* ->
    s_ca = af~^T Tt ae~ with Tt a per-sample gathered 4x4x4 table
    (bias row/col augmented).
  - h1 = sum_i D_i z_i + sum_i B_i ao_e_i + m_tab[map] with
    z_i = sum_j a_ca[j,i] ao_e_j (Mv and fc1 blocks folded into D_i).
  - counter influence: unscaled pair-sum table P2[e1,e2] + single row,
    scaled by 1/valid on device via a diagonal matmul.
"""

import numpy as np

import concourse.bass as bass
import concourse.bacc as bacc
import concourse.tile as tile
import concourse.mybir as mybir
from concourse.masks import make_identity

F32 = mybir.dt.float32
BF16 = mybir.dt.bfloat16
I32 = mybir.dt.int32

B_FULL = 65536
NCORES = 8
E, NH, DH, S = 32, 4, 8, 3
NB, NM, H = 512, 128, 128
AOP = mybir.AluOpType
AFT = mybir.ActivationFunctionType

# per-sample record layout (bf16 elements)
OFF_SEA, OFF_SFA, OFF_T, OFF_VE, OFF_MR, REC = 0, 36, 72, 136, 232, 360

# ---------------------------------------------------------------------------
# host-side precompute (input-independent tables)
# ---------------------------------------------------------------------------


def host_precompute(inp):
    f32 = np.float32
    emb = np.asarray(inp['brawler_emb'], f32)
    pos_w = np.asarray(inp['pos_w'], f32)
    pos_b = np.asarray(inp['pos_b'], f32)
    pos_emb = np.arange(S, dtype=f32)[:, None] * pos_w[None, :, 0] + pos_b

    def split_in(w, b):
        w = np.asarray(w, f32)
        b = np.asarray(b, f32)
        return (w[:E], w[E:2 * E], w[2 * E:], b[:E], b[E:2 * E], b[2 * E:])

    Wq_ea, Wk_ea, Wv_ea, bq_ea, bk_ea, bv_ea = split_in(inp['ea_in_w'], inp['ea_in_b'])
    Wq_fa, Wk_fa, Wv_fa, bq_fa, bk_fa, bv_fa = split_in(inp['fa_in_w'], inp['fa_in_b'])
    Wq_ca, Wk_ca, Wv_ca, bq_ca, bk_ca, bv_ca = split_in(inp['ca_in_w'], inp['ca_in_b'])
    Wout_ea = np.asarray(inp['ea_out_w'], f32)
    bout_ea = np.asarray(inp['ea_out_b'], f32)
    Wout_fa = np.asarray(inp['fa_out_w'], f32)
    bout_fa = np.asarray(inp['fa_out_b'], f32)
    Wout_ca = np.asarray(inp['ca_out_w'], f32)

    t_ea = emb[None] + pos_emb[:, None]                  # (3, 512, 32)

    def hdot(A, B):
        return np.einsum('...hd,...hd->...h',
                         A.reshape(*A.shape[:-1], NH, DH),
                         B.reshape(*B.shape[:-1], NH, DH))

    q_ea = t_ea @ Wq_ea.T + bq_ea
    k_ea = t_ea @ Wk_ea.T + bk_ea
    T_EA = hdot(q_ea[:, :, None, None], k_ea[None, None])    # (3,512,3,512,4)
    q_fa = emb @ Wq_fa.T + bq_fa
    k_fa = emb @ Wk_fa.T + bk_fa
    T_FA = hdot(q_fa[:, None], k_fa[None])                   # (512, 512, 4)

    v_ea = t_ea @ Wv_ea.T                                    # (3, 512, 32)
    v_fa = emb @ Wv_fa.T                                     # (512, 32)

    Mq = Wq_ca @ Wout_fa
    bq_f = Mq @ bv_fa + Wq_ca @ bout_fa + bq_ca
    Mk = Wk_ca @ Wout_ea
    bk_f = Mk @ bv_ea + Wk_ca @ bout_ea + bk_ca
    Mv = Wv_ca @ Wout_ea

    TqA = np.concatenate([v_fa @ Mq.T, bq_f[None]], 0)       # (513, 32)
    TkA = np.concatenate([(v_ea @ Mk.T).reshape(3 * NB, E), bk_f[None]], 0)
    TCA = hdot(TqA[:, None], TkA[None])                      # (513, 1537, 4)

    fc1_w = np.asarray(inp['fc1_w'], f32)
    A_ca = fc1_w[:, 0:96].reshape(H, 3, E)
    A_ea = fc1_w[:, 96:192].reshape(H, 3, E)
    A_m = fc1_w[:, 192:224]
    m_tab = np.asarray(inp['map_emb'], f32) @ A_m.T          # (128, 128)

    # db[i,:,0] = D_i.T (Mv folded), db[i,:,1] = B_i.T
    db = np.zeros((3, 32, 2, 128), f32)
    for i in range(3):
        db[i, :, 0] = ((A_ca[:, i] @ Wout_ca) @ Mv).T
        db[i, :, 1] = (A_ea[:, i] @ Wout_ea).T

    counter = np.asarray(inp['counter_matrix'], f32)
    nz = (np.arange(NB) != 0).astype(f32)
    cn = nz[:, None] * counter                               # (512, 512)

    W3aug = np.concatenate([np.asarray(inp['fc3_w'], f32).T,
                            np.asarray(inp['fc3_b'], f32)[None, :]], 0)

    return dict(
        T_EA=T_EA, T_FA=T_FA, TCA=TCA, v_ea=v_ea, v_fa=v_fa, m_tab=m_tab,
        cn=cn, dbT=db.reshape(96, 2, 128), w2T=np.asarray(inp['fc2_w'], f32).T,
        w3aug=W3aug,
        bn1_g=np.asarray(inp['bn1_g'], f32), bn1_b=np.asarray(inp['bn1_b'], f32),
        bn2_g=np.asarray(inp['bn2_g'], f32), bn2_b=np.asarray(inp['bn2_b'], f32),
    )


# ---------------------------------------------------------------------------
# device kernel
# ---------------------------------------------------------------------------


def build_nc(b, n_cores, with_collective=True, stage=5):
    assert b % 1024 == 0
    nc = bacc.Bacc("TRN2", target_bir_lowering=False, debug=False,
                   num_devices=n_cores)

    G1 = 16                    # P1 chunk = 2048 samples
    CH1 = G1 * 128
    NCH1 = b // CH1
    G3 = 8                     # P3 chunk = 1024 samples
    CH3 = G3 * 128
    NCH3 = b // CH3
    NGG = b // 128             # total 128-sample groups
    btot = float(b * (n_cores if with_collective else 1))
    rsq = float(1.0 / np.sqrt(DH))

    dt_i = nc.dram_tensor
    xrec = dt_i("xrec", (128, NGG * REC), BF16, kind="ExternalInput")
    ct_in = dt_i("ct", (128, NGG * 2 * NB), mybir.dt.float8e4,
                 kind="ExternalInput")
    id8_in = dt_i("id8", (128, 128), mybir.dt.float8e4, kind="ExternalInput")
    dbT = dt_i("dbT", (96, 2, 128), BF16, kind="ExternalInput")
    w2T = dt_i("w2T", (128, 64), BF16, kind="ExternalInput")
    w3aug = dt_i("w3aug", (65, NB), BF16, kind="ExternalInput")
    bn_g1 = dt_i("bn_g1", (H, 1), F32, kind="ExternalInput")
    bn_b1 = dt_i("bn_b1", (H, 1), F32, kind="ExternalInput")
    bn_g2 = dt_i("bn_g2", (64, 1), F32, kind="ExternalInput")
    bn_b2 = dt_i("bn_b2", (64, 1), F32, kind="ExternalInput")
    out_t = dt_i("out", (b, NB), BF16, kind="ExternalOutput")

    import contextlib
    with tile.TileContext(nc) as tc, contextlib.ExitStack() as ctx:
        singles = ctx.enter_context(tc.tile_pool(name="singles", bufs=1))
        dram = ctx.enter_context(tc.tile_pool(name="dram", bufs=1, space="DRAM"))

        ident = singles.tile([128, 128], BF16)
        make_identity(nc, ident[:, :])

        def load(name, shape, dtype, src):
            t = singles.tile(shape, dtype, tag="c_" + name)
            nc.sync.dma_start(out=t[...], in_=src)
            return t

        c_dbT = load("dbT", [96, 2, 128], BF16, dbT[:, :, :])
        c_w2T = load("w2T", [128, 64], BF16, w2T[:, :])
        c_w3aug = load("w3aug", [65, NB], BF16, w3aug[:, :])
        c_g1 = load("g1", [H, 1], F32, bn_g1[:, :])
        c_b1 = load("b1", [H, 1], F32, bn_b1[:, :])
        c_g2 = load("g2", [64, 1], F32, bn_g2[:, :])
        c_b2 = load("b2", [64, 1], F32, bn_b2[:, :])
        c_id8 = load("id8", [128, 128], mybir.dt.float8e4, id8_in[:, :])

        if with_collective:
            warm_in = dram.tile([1, 2], F32, tag="warm_in")
            warm_out = nc.dram_tensor("warm_out", (1, 2), F32,
                                      kind="Internal", addr_space="Shared")
            wz = singles.tile([1, 2], F32, tag="warm_z")
            nc.vector.memset(wz[...], 0.0)
            nc.sync.dma_start(out=warm_in[:, :], in_=wz[...])
            nc.gpsimd.collective_compute(
                "AllReduce", AOP.add,
                replica_groups=[list(range(n_cores))],
                ins=[warm_in[:, :].opt()], outs=[warm_out[:, :].opt()])

        h1 = singles.tile([128, b], BF16)
        h2 = singles.tile([64, b], BF16)
        a2aug = singles.tile([65, b], BF16)
        nc.vector.memset(a2aug[64:65, :], 1.0)
        # per-512-chunk BN stat partials (sum / sum-of-squares)
        s1p = singles.tile([128, b // 512], F32)
        q1p = singles.tile([128, b // 512], F32)
        s2p = singles.tile([64, b // 1024], F32)

        # --- P1: softmax / AV / ca-bilinear chain + h1 ---------------------
        with tc.tile_pool(name="attn", bufs=1) as atp, \
             tc.tile_pool(name="expool", bufs=2) as expool, \
             tc.tile_pool(name="gath", bufs=3) as gath, \
             tc.tile_pool(name="ao", bufs=2) as aopool, \
             tc.tile_pool(name="stag", bufs=2) as stag, \
             tc.tile_pool(name="ps_t", bufs=1, space="PSUM") as ps_t, \
             tc.tile_pool(name="ps_h1", bufs=2, space="PSUM") as ps_h1:
            for ch in range(NCH1):
                xr = gath.tile([128, G1, REC], BF16, tag="xr")
                nc.sync.dma_start(out=xr[...],
                                  in_=xrec[:, ch * G1 * REC:(ch + 1) * G1 * REC])

                # ea+fa softmax from gathered scores (layout w, j(key), i, h)
                e_b = expool.tile([128, G1, 2, 3, 12], F32, tag="e_b")
                nc.scalar.activation(
                    out=e_b.rearrange("p g w j x -> p g (w j x)"),
                    in_=xr[:, :, 0:72], func=AFT.Exp, scale=rsq)
                den = atp.tile([128, G1, 2, 12], F32, tag="den")
                nc.vector.tensor_tensor(out=den, in0=e_b[:, :, :, 0],
                                        in1=e_b[:, :, :, 1], op=AOP.add)
                den2 = atp.tile([128, G1, 2, 12], F32, tag="den2")
                nc.vector.tensor_tensor(out=den2, in0=den, in1=e_b[:, :, :, 2],
                                        op=AOP.add)
                r_b = atp.tile([128, G1, 2, 12], F32, tag="r_b")
                rs = atp.tile([128, G1, 2, 12], F32, tag="rs")
                nc.vector.reciprocal_approx_accurate(
                    out=r_b.rearrange("p g w x -> p (g w x)"),
                    in_=den2.rearrange("p g w x -> p (g w x)"),
                    scratch=rs.rearrange("p g w x -> p (g w x)"))
                a_b = atp.tile([128, G1, 2, 3, 12], BF16, tag="a_b")
                for w in range(2):
                    nc.vector.tensor_tensor(
                        out=a_b[:, :, w], in0=e_b[:, :, w],
                        in1=r_b[:, :, w].unsqueeze(2)
                        .to_broadcast([128, G1, 3, 12]),
                        op=AOP.mult)

                # AV for ea and fa: ao[i] = sum_j a[j,i] (x) v[j]
                ao_e = aopool.tile([128, G1, 3, E], BF16, tag="ao_e")

                def attn_av(w, a_t, v_of_j, ao):
                    # v_of_j(j) -> [128, G1, NH, DH]-broadcastable AP
                    avs = []
                    for j in range(3):
                        av = atp.tile([128, G1, 3, NH, DH], BF16,
                                      tag=f"av{w}{j}")
                        v_in = v_of_j(j)
                        for i in range(3):
                            a_in = a_t[:, :, j, 4 * i:4 * (i + 1)] \
                                .unsqueeze(3).to_broadcast([128, G1, NH, DH])
                            nc.vector.tensor_tensor(out=av[:, :, i],
                                                    in0=a_in, in1=v_in,
                                                    op=AOP.mult)
                        avs.append(av)
                    s01 = atp.tile([128, G1, 3, NH, DH], BF16, tag=f"s01{w}")
                    nc.vector.tensor_tensor(out=s01[...], in0=avs[0][...],
                                            in1=avs[1][...], op=AOP.add)
                    nc.vector.tensor_tensor(
                        out=ao.rearrange("p g i (h d) -> p g i h d", d=DH),
                        in0=s01[...], in1=avs[2][...], op=AOP.add)

                attn_av(0, a_b[:, :, 0],
                        lambda j: xr[:, :, OFF_VE + E * j:OFF_VE + E * (j + 1)]
                        .rearrange("p g (h d) -> p g h d", d=DH), ao_e)

                # ca bilinear scores: s[j,i,h] = af~_i^T Tt ae~_j
                # U[i,m,h] = sum_l af[l,i,h] Tt[l,m,h]  (+ bias row l'=3)
                us = []
                for l in range(3):
                    u = atp.tile([128, G1, 3, 4, NH], BF16, tag=f"u{l}")
                    tl = xr[:, :, OFF_T + 16 * l:OFF_T + 16 * (l + 1)] \
                        .rearrange("p g (m h) -> p g m h", h=NH)
                    for i in range(3):
                        af = a_b[:, :, 1, l, 4 * i:4 * (i + 1)] \
                            .unsqueeze(2).to_broadcast([128, G1, 4, NH])
                        nc.vector.tensor_tensor(out=u[:, :, i], in0=af,
                                                in1=tl, op=AOP.mult)
                    us.append(u)
                t3b = xr[:, :, OFF_T + 48:OFF_T + 64].rearrange(
                    "p g (m h) -> p g m h", h=NH).unsqueeze(2) \
                    .to_broadcast([128, G1, 3, 4, NH])
                us1 = atp.tile([128, G1, 3, 4, NH], BF16, tag="us1")
                us2 = atp.tile([128, G1, 3, 4, NH], BF16, tag="us2")
                U = atp.tile([128, G1, 3, 4, NH], BF16, tag="U")
                nc.vector.tensor_tensor(out=us1[...], in0=us[0][...],
                                        in1=us[1][...], op=AOP.add)
                nc.vector.tensor_tensor(out=us2[...], in0=us[2][...], in1=t3b,
                                        op=AOP.add)
                nc.vector.tensor_tensor(out=U[...], in0=us1[...], in1=us2[...],
                                        op=AOP.add)

                # s_ca[j,i,h] = sum_m U[i,m,h] ae[m,j,h]  (+ bias col m'=3)
                sc = atp.tile([128, G1, 3, 3, NH], BF16, tag="sc")
                for i in range(3):
                    vs = []
                    for m in range(3):
                        v = atp.tile([128, G1, 3, NH], BF16, tag=f"vv{i}{m}")
                        ui = U[:, :, i, m, :].unsqueeze(2) \
                            .to_broadcast([128, G1, 3, NH])
                        ae = a_b[:, :, 0, m].rearrange(
                            "p g (j2 h) -> p g j2 h", h=NH)
                        nc.vector.tensor_tensor(out=v[...], in0=ui, in1=ae,
                                                op=AOP.mult)
                        vs.append(v)
                    u3 = U[:, :, i, 3, :].unsqueeze(2) \
                        .to_broadcast([128, G1, 3, NH])
                    vs1 = atp.tile([128, G1, 3, NH], BF16, tag=f"vs1{i}")
                    vs2 = atp.tile([128, G1, 3, NH], BF16, tag=f"vs2{i}")
                    nc.vector.tensor_tensor(out=vs1[...], in0=vs[0][...],
                                            in1=vs[1][...], op=AOP.add)
                    nc.vector.tensor_tensor(out=vs2[...], in0=vs[2][...],
                                            in1=u3, op=AOP.add)
                    nc.vector.tensor_tensor(out=sc[:, :, :, i, :],
                                            in0=vs1[...], in1=vs2[...],
                                            op=AOP.add)

                # ca softmax
                e_c = expool.tile([128, G1, 3, 12], F32, tag="e_c")
                nc.scalar.activation(
                    out=e_c.rearrange("p g j x -> p g (j x)"),
                    in_=sc.rearrange("p g j i h -> p g (j i h)"),
                    func=AFT.Exp, scale=rsq)
                dc = atp.tile([128, G1, 12], F32, tag="dc")
                nc.vector.tensor_tensor(out=dc, in0=e_c[:, :, 0],
                                        in1=e_c[:, :, 1], op=AOP.add)
                dc2 = atp.tile([128, G1, 12], F32, tag="dc2")
                nc.vector.tensor_tensor(out=dc2, in0=dc, in1=e_c[:, :, 2],
                                        op=AOP.add)
                r_c = atp.tile([128, G1, 12], F32, tag="r_c")
                rcs = atp.tile([128, G1, 12], F32, tag="rcs")
                nc.vector.reciprocal_approx_accurate(
                    out=r_c.rearrange("p g x -> p (g x)"),
                    in_=dc2.rearrange("p g x -> p (g x)"),
                    scratch=rcs.rearrange("p g x -> p (g x)"))
                a_c = atp.tile([128, G1, 3, 12], BF16, tag="a_c")
                nc.vector.tensor_tensor(
                    out=a_c, in0=e_c,
                    in1=r_c.unsqueeze(2).to_broadcast([128, G1, 3, 12]),
                    op=AOP.mult)

                # z AV over ao_e
                z = aopool.tile([128, G1, 3, E], BF16, tag="z")
                attn_av(2, a_c[:, :, :, :],
                        lambda j: ao_e[:, :, j].rearrange(
                            "p g (h d) -> p g h d", d=DH), z)

                # per 512-sample sub-chunk: transposes + h1 accumulation
                for sub in range(G1 // 4):
                    g0 = sub * 4
                    sidx = ch * (G1 // 4) + sub
                    col0 = sidx * 512
                    aoeT_ps = ps_t.tile([96, 512], BF16, tag="aoeT")
                    zT_ps = ps_t.tile([96, 512], BF16, tag="zT")
                    for t in range(4):
                        nc.tensor.transpose(
                            aoeT_ps[:, t * 128:(t + 1) * 128],
                            ao_e[:, g0 + t].rearrange("p i d -> p (i d)"),
                            ident[:, :])
                        nc.tensor.transpose(
                            zT_ps[:, t * 128:(t + 1) * 128],
                            z[:, g0 + t].rearrange("p i d -> p (i d)"),
                            ident[:, :])
                    aoeT = stag.tile([96, 512], BF16, tag="aoeT_s")
                    zT = stag.tile([96, 512], BF16, tag="zT_s")
                    nc.scalar.activation(out=aoeT[...], in_=aoeT_ps[...],
                                         func=AFT.Copy)
                    nc.scalar.activation(out=zT[...], in_=zT_ps[...],
                                         func=AFT.Copy)

                    h1_ps = ps_h1.tile([128, 512], F32, tag="h1ps")
                    nc.tensor.matmul(h1_ps[...], c_dbT[:, 0, :], zT[...],
                                     start=True, stop=False)
                    nc.tensor.matmul(h1_ps[...], c_dbT[:, 1, :], aoeT[...],
                                     start=False, stop=False)
                    for t in range(4):
                        nc.tensor.matmul(
                            h1_ps[:, t * 128:(t + 1) * 128],
                            xr[:, g0 + t, OFF_MR:OFF_MR + 128],
                            ident[:, :], start=False, stop=(t == 3))
                    nc.scalar.activation(out=h1[:, col0:col0 + 512],
                                         in_=h1_ps[...], func=AFT.Copy,
                                         accum_out=s1p[:, sidx:sidx + 1])
                    sq = stag.tile([128, 512], F32, tag="sq")
                    nc.scalar.activation(out=sq[...], in_=h1_ps[...],
                                         func=AFT.Square,
                                         accum_out=q1p[:, sidx:sidx + 1])

        # --- BN (exact global stats; partials already accumulated) ---------
        def bn_stats(parts, sp, qp, g_col, b_col, cc_name):
            sq01 = singles.tile([parts, 2], F32, tag=cc_name + "_sq01")
            nc.vector.tensor_reduce(out=sq01[:, 0:1], in_=sp,
                                    axis=mybir.AxisListType.X, op=AOP.add)
            nc.vector.tensor_reduce(out=sq01[:, 1:2], in_=qp,
                                    axis=mybir.AxisListType.X, op=AOP.add)
            if with_collective:
                cc_in = dram.tile([parts, 2], F32, tag=cc_name + "_in")
                cc_out = nc.dram_tensor(cc_name + "_out", (parts, 2), F32,
                                        kind="Internal", addr_space="Shared")
                nc.sync.dma_start(out=cc_in[:, :], in_=sq01[...])
                nc.gpsimd.collective_compute(
                    "AllReduce", AOP.add,
                    replica_groups=[list(range(n_cores))],
                    ins=[cc_in[:, :].opt()], outs=[cc_out[:, :].opt()])
                sq = singles.tile([parts, 2], F32, tag=cc_name + "_sq")
                nc.sync.dma_start(out=sq[...], in_=cc_out[:, :])
                s1g, q1g = sq[:, 0:1], sq[:, 1:2]
            else:
                s1g, q1g = sq01[:, 0:1], sq01[:, 1:2]
            mean = singles.tile([parts, 1], F32, tag=cc_name + "_mean")
            nc.vector.tensor_scalar_mul(mean[...], s1g, 1.0 / btot)
            msq = singles.tile([parts, 1], F32, tag=cc_name + "_msq")
            nc.vector.tensor_scalar_mul(msq[...], q1g, 1.0 / btot)
            m2 = singles.tile([parts, 1], F32, tag=cc_name + "_m2")
            nc.vector.tensor_tensor(out=m2[...], in0=mean[...], in1=mean[...],
                                    op=AOP.mult)
            var = singles.tile([parts, 1], F32, tag=cc_name + "_var")
            nc.vector.tensor_tensor(out=var[...], in0=msq[...], in1=m2[...],
                                    op=AOP.subtract)
            eps = singles.tile([parts, 1], F32, tag=cc_name + "_eps")
            nc.vector.memset(eps[...], 1e-5)
            std = singles.tile([parts, 1], F32, tag=cc_name + "_std")
            nc.scalar.activation(out=std[...], in_=var[...], func=AFT.Sqrt,
                                 bias=eps[...])
            rstd = singles.tile([parts, 1], F32, tag=cc_name + "_rstd")
            nc.vector.reciprocal(out=rstd[...], in_=std[...])
            scale = singles.tile([parts, 1], F32, tag=cc_name + "_scale")
            nc.vector.tensor_tensor(out=scale[...], in0=g_col[...],
                                    in1=rstd[...], op=AOP.mult)
            mscale = singles.tile([parts, 1], F32, tag=cc_name + "_ms")
            nc.vector.tensor_tensor(out=mscale[...], in0=mean[...],
                                    in1=scale[...], op=AOP.mult)
            nbias = singles.tile([parts, 1], F32, tag=cc_name + "_nb")
            nc.vector.tensor_tensor(out=nbias[...], in0=b_col[...],
                                    in1=mscale[...], op=AOP.subtract)
            return scale, nbias

        sc1, nb1 = bn_stats(128, s1p[:, :], q1p[:, :], c_g1, c_b1, "cc1")

        with tc.tile_pool(name="ps_h2", bufs=2, space="PSUM") as ps_h2, \
             tc.tile_pool(name="h2st", bufs=2) as h2st:
            for sc2 in range(b // 1024):
                a1c = h2st.tile([128, 1024], BF16, tag="a1c")
                nc.scalar.activation(out=a1c[...],
                                     in_=h1[:, sc2 * 1024:(sc2 + 1) * 1024],
                                     func=AFT.Relu, bias=nb1[...],
                                     scale=sc1[...])
                h2_ps = ps_h2.tile([64, 2, 512], F32, tag="h2ps")
                for k in range(2):
                    nc.tensor.matmul(h2_ps[:, k], c_w2T[:, :],
                                     a1c[:, k * 512:(k + 1) * 512],
                                     start=True, stop=True)
                nc.scalar.activation(out=h2[:, sc2 * 1024:(sc2 + 1) * 1024],
                                     in_=h2_ps.rearrange("p k x -> p (k x)"),
                                     func=AFT.Copy,
                                     accum_out=s2p[:, sc2:sc2 + 1])

        # one-pass sum-of-squares for h2 (a2aug rows are free until the
        # BN2 apply overwrites them)
        q2c = singles.tile([64, 1], F32, tag="q2c")
        nc.scalar.activation(out=a2aug[0:64, :], in_=h2[:, :], func=AFT.Square,
                             accum_out=q2c[...])
        sc2_, nb2_ = bn_stats(64, s2p[:, :], q2c[:, :], c_g2, c_b2, "cc2")
        nc.scalar.activation(out=a2aug[0:64, :], in_=h2[:, :], func=AFT.Relu,
                             bias=nb2_[...], scale=sc2_[...])

        # --- P3: fc3 + counter + writeback ---------------------------------
        with tc.tile_pool(name="ps_o", bufs=2, space="PSUM") as ps_o, \
             tc.tile_pool(name="ctp", bufs=4) as ctpool, \
             tc.tile_pool(name="ost", bufs=2) as ost:
            out_r = out_t[:, :].rearrange("(g p) n -> p g n", p=128)
            for ch in range(NCH3):
                ct = ctpool.tile([128, G3, 2, NB], mybir.dt.float8e4, tag="ct")
                c0 = ch * G3 * 2 * NB
                nc.sync.dma_start(out=ct[...], in_=ct_in[:, c0:c0 + G3 * 2 * NB])
                ostg = ost.tile([128, G3, NB], BF16, tag="ostg")
                for g2 in range(G3 // 2):
                    o_ps = ps_o.tile([128, 2, NB], F32, tag="ops")
                    for k in range(2):
                        g = g2 * 2 + k
                        gg = ch * G3 + g
                        nc.tensor.matmul(o_ps[:, k],
                                         a2aug[:, gg * 128:(gg + 1) * 128],
                                         c_w3aug[:, :], start=True, stop=False)
                        nc.tensor.matmul(o_ps[:, k], c_id8[...], ct[:, g, 0, :],
                                         start=False, stop=False)
                        nc.tensor.matmul(o_ps[:, k], c_id8[...], ct[:, g, 1, :],
                                         start=False, stop=True)
                    nc.scalar.activation(
                        out=ostg[:, g2 * 2:(g2 + 1) * 2],
                        in_=o_ps.rearrange("p k x -> p (k x)"), func=AFT.Copy)
                nc.sync.dma_start(out=out_r[:, ch * G3:(ch + 1) * G3, :],
                                  in_=ostg[...])

    nc.compile()
    return nc


# ---------------------------------------------------------------------------
# host wrapper
# ---------------------------------------------------------------------------

_NC_CACHE = {}


def make_core_inputs(inputs, pc, b, n_cores):
    import ml_dtypes
    bf16 = ml_dtypes.bfloat16
    friends = np.asarray(inputs['friends'], np.int64)
    enemies = np.asarray(inputs['enemies'], np.int64)
    map_idx = np.asarray(inputs['map_idx'], np.int64)
    n = friends.shape[0]

    # bf16 tables: host gathers are then raw row copies
    TEA = pc['T_EA'].reshape(3 * NB, 3 * NB, NH).astype(bf16)
    TFA = pc['T_FA'].astype(bf16)
    TCA = pc['TCA'].astype(bf16)
    v_ea = pc['v_ea'].astype(bf16)
    m_tab = pc['m_tab'].astype(bf16)
    cn = pc['cn'].astype(np.float32)
    P2f = cn[:, None, :] + cn[None, :, :]

    # per-sample record [S_ea | S_fa | Tt | v_e | v_f | mrow]
    rec = np.empty((n, REC), bf16)
    ei = np.arange(3)[None] * NB + enemies                  # (n, 3) token idx
    # scores stored (j_key, i_query, h): TEA[A,B][b,x,y] = s(query=y, key=x)
    rec[:, OFF_SEA:OFF_SFA] = TEA[ei[:, None, :], ei[:, :, None]].reshape(n, 36)
    rec[:, OFF_SFA:OFF_T] = TFA[friends[:, None, :], friends[:, :, None]] \
        .reshape(n, 36)
    la = np.concatenate([friends, np.full((n, 1), NB)], 1)
    mb = np.concatenate([ei, np.full((n, 1), 3 * NB)], 1)
    rec[:, OFF_T:OFF_VE] = TCA[la[:, :, None], mb[:, None, :]].reshape(n, 64)
    rec[:, OFF_VE:OFF_MR] = v_ea[np.arange(3)[None], enemies].reshape(n, 96)
    rec[:, OFF_MR:] = m_tab[map_idx[:, 0]]

    f8 = ml_dtypes.float8_e4m3
    valid = (enemies != 0).sum(1)
    vi = np.maximum(valid, 1) - 1                       # 0..2 scale version
    P2s = np.empty((3, NB * NB, NB), f8)
    cns = np.empty((3, NB, NB), f8)
    for v in range(3):
        P2s[v] = (P2f * (1.0 / (v + 1))).reshape(NB * NB, NB).astype(f8)
        cns[v] = (cn * (1.0 / (v + 1))).astype(f8)
    ctg = np.empty((n, 2, NB), f8)
    ctg[:, 0] = P2s[vi, enemies[:, 0] * NB + enemies[:, 1]]
    ctg[:, 1] = cns[vi, enemies[:, 2]]

    shared = dict(
        id8=np.eye(128, dtype=f8),
        dbT=pc['dbT'].astype(bf16), w2T=pc['w2T'].astype(bf16),
        w3aug=pc['w3aug'].astype(bf16),
        bn_g1=pc['bn1_g'].reshape(-1, 1).astype(np.float32),
        bn_b1=pc['bn1_b'].reshape(-1, 1).astype(np.float32),
        bn_g2=pc['bn2_g'].reshape(-1, 1).astype(np.float32),
        bn_b2=pc['bn2_b'].reshape(-1, 1).astype(np.float32),
    )

    in_maps = []
    for c in range(n_cores):
        lo, hi = c * b, (c + 1) * b
        m = dict(shared)
        m['xrec'] = np.ascontiguousarray(
            rec[lo:hi].reshape(b // 128, 128, REC).transpose(1, 0, 2)
        ).reshape(128, -1)
        m['ct'] = np.ascontiguousarray(
            ctg[lo:hi].reshape(b // 128, 128, 2 * NB).transpose(1, 0, 2)
        ).reshape(128, -1)
        in_maps.append(m)
    return in_maps


def kernel(**inputs):
    from concourse.bass_utils import run_bass_kernel_spmd
    b = B_FULL // NCORES
    pc = host_precompute(inputs)
    key = (b, NCORES)
    if key not in _NC_CACHE:
        _NC_CACHE[key] = build_nc(b, NCORES, with_collective=True)
    nc = _NC_CACHE[key]
    in_maps = make_core_inputs(inputs, pc, b, NCORES)
    res = run_bass_kernel_spmd(nc, in_maps, core_ids=list(range(NCORES)))
    out = np.concatenate([np.asarray(r['out'], np.float32)
                          for r in res.results], 0)
    return out
